# revision 4
# baseline (speedup 1.0000x reference)
"""GQA attention kernel for 8 Trainium2 NeuronCores.

Sharding: sequence-parallel. Core c handles batch b = c//4 and query rows
[(c%4)*512, (c%4+1)*512) of that batch. Each core computes the full K/V
projection for its batch (duplicated 4x) so there are no collectives; the
host just concatenates the 8 output row-blocks.

All activations are kept feature-major on-chip. The host pre-transposes
q/k/v (and un-transposes the output), so the kernel contains NO PE
transposes -- every TensorE instruction is a productive matmul:

  qT  [e, sq]   <- DMA (host-transposed)
  QT  [d,h,sq]  <- Wq.T @ qT        (per 512-col quarter, 4 PSUM banks)
  KT  [d,g,skv] <- Wk.T @ kT        (per 512-row skv chunk)
  Vn  [skv,kv]  <- vT.T @ Wv        (vT slice stationary, Wv moving)
  per head h (group g = h//4), per skv chunk c (128 rows):
    scoresT[c] = KT[g,c].T @ QT[h]            (PSUM)
    PT[c] = exp(scoresT*scale + maskbias)     (ACT, PSUM->SBUF, fp32r)
    rowsum += ones.T @ PT[c]   ;  OT[h] += Vn[c,g].T @ PT[c]   (PSUM acc)
  (rowsum/AV for chunk c are emitted after scores chunk c+1 so the PE
   never waits on the ACT exp latency)
  OT[h] *= 1/rowsum (broadcast via rank-1 matmul), YT = Wo.T @ OT -> DMA.

float32r is used for every matmul operand (full fp32 precision at bf16
streaming rate when the moving dim >= 256).
"""

import os
import sys

sys.path.insert(0, "/opt/trn_rl_repo")
if os.environ.get("JAX_PLATFORMS") == "cpu":
    del os.environ["JAX_PLATFORMS"]
os.environ.setdefault("MYCRO_LOCAL_CACHE", "1")

from contextlib import ExitStack

import numpy as np

import concourse.bass as bass
import concourse.bacc as bacc
import concourse.mybir as mybir
import concourse.tile as tile

P = 128
E = 2048          # embed dim
SQ = 512          # query rows per core
SKV = 2048        # kv sequence length
KV = 512          # kv projection width (4 kv heads * 128)
H = 16            # query heads
nE = E // P       # 16
nKV = SKV // P    # 16
SC = 1.0 / float(128.0) ** 0.5
B, S = 2, 2048
N_CORES = 8

F32 = mybir.dt.float32
R = mybir.dt.float32r
AF = mybir.ActivationFunctionType


def _round_f32r(x):
    """Round fp32 to the fp32r-representable subset (8 explicit mantissa bits,
    round-to-nearest-even) so DMA'd bytes match what the PE streams."""
    u = np.ascontiguousarray(x, dtype=np.float32).view(np.uint32).copy()
    half = np.uint32(1 << 14)
    lsb = (u >> np.uint32(15)) & np.uint32(1)
    u = (u + half - np.uint32(1) + lsb) & np.uint32(0xFFFF8000)
    return u.view(np.float32)


def build_nc():
    nc = bacc.Bacc(target_bir_lowering=False)

    qt_d = nc.dram_tensor("qt", [E, SQ], R, kind="ExternalInput")
    kt_d = nc.dram_tensor("kt", [E, SKV], R, kind="ExternalInput")
    vt_d = nc.dram_tensor("vt", [E, SKV], R, kind="ExternalInput")
    m_d = nc.dram_tensor("m", [SKV], F32, kind="ExternalInput")
    wq_d = nc.dram_tensor("wq", [E, E], R, kind="ExternalInput")
    wk_d = nc.dram_tensor("wk", [E, KV], R, kind="ExternalInput")
    wv_d = nc.dram_tensor("wv", [E, KV], R, kind="ExternalInput")
    wo_d = nc.dram_tensor("wo", [E, E], R, kind="ExternalInput")
    yt_d = nc.dram_tensor("yt", [E, SQ], F32, kind="ExternalOutput")

    with ExitStack() as ctx:
        tc = ctx.enter_context(tile.TileContext(nc))
        consts = ctx.enter_context(tc.tile_pool(name="consts", bufs=1))
        wpool = ctx.enter_context(tc.tile_pool(name="wpool", bufs=4))
        apool = ctx.enter_context(tc.tile_pool(name="apool", bufs=4))
        bigq = ctx.enter_context(tc.tile_pool(name="bigq", bufs=1))
        bigk = ctx.enter_context(tc.tile_pool(name="bigk", bufs=1))
        bigv = ctx.enter_context(tc.tile_pool(name="bigv", bufs=1))
        bigqo = ctx.enter_context(tc.tile_pool(name="bigqo", bufs=1))
        ptp = ctx.enter_context(tc.tile_pool(name="ptp", bufs=2))
        small = ctx.enter_context(tc.tile_pool(name="small", bufs=2))
        psmm = ctx.enter_context(tc.tile_pool(name="psmm", bufs=4, space="PSUM"))
        psra = ctx.enter_context(tc.tile_pool(name="psra", bufs=3, space="PSUM"))
        ystg = ctx.enter_context(tc.tile_pool(name="ystg", bufs=4))

        # ---- constants ----
        ones_f = consts.tile([P, 1], F32, tag="ones_f")
        nc.vector.memset(ones_f, 1.0)
        ones_col = consts.tile([P, 1], R, tag="ones")
        nc.vector.tensor_copy(ones_col, ones_f)
        ones_row = consts.tile([1, P], F32, tag="ones_r")
        nc.vector.memset(ones_row, 1.0)
        mask_sb = consts.tile([P, nKV], F32, tag="msk")
        nc.sync.dma_start(out=mask_sb, in_=m_d.rearrange("(a b) -> b a", b=P))
        bias_sb = consts.tile([P, nKV], F32, tag="bias")
        # (mask - 1) * 1e9 : zero where mask==1, -1e9 where mask==0
        nc.scalar.activation(bias_sb, mask_sb, AF.Copy, bias=-1e9, scale=1e9)

        # ---- phase 1: qT [P(e), nE, SQ] via one strided DMA ----
        qT = bigqo.tile([P, nE, SQ], R, tag="qo")
        nc.sync.dma_start(out=qT, in_=qt_d.rearrange("(a b) c -> b a c", b=P))

        # ---- phase 2: Qproj -> QT [P(d), H, SQ] ----
        QT = bigq.tile([P, H, SQ], R, tag="qt")
        for mq in range(4):
            pss = [psmm.tile([P, SQ], F32, tag="mm", name=f"ps{_i}") for _i in range(4)]
            for e in range(nE):
                wt = wpool.tile([P, 512], R, tag="w")
                nc.sync.dma_start(
                    out=wt, in_=wq_d[e * 128:(e + 1) * 128, mq * 512:(mq + 1) * 512]
                )
                for j in range(4):
                    nc.tensor.matmul(
                        pss[j], wt[:, j * 128:(j + 1) * 128], qT[:, e, :],
                        start=(e == 0), stop=(e == nE - 1), skip_group_check=True,
                    )
            for j in range(4):
                nc.vector.tensor_copy(QT[:, mq * 4 + j, :], pss[j])

        # ---- phase 3: Kproj -> KT [P(d), G, SKV] ----
        KT = bigk.tile([P, 4, SKV], R, tag="kt")
        for cs in range(4):
            pss = [psmm.tile([P, 512], F32, tag="mm", name=f"ps{_i}") for _i in range(4)]
            for e in range(nE):
                kt = apool.tile([P, 512], R, tag="a")
                nc.sync.dma_start(
                    out=kt, in_=kt_d[e * 128:(e + 1) * 128, cs * 512:(cs + 1) * 512]
                )
                wt = wpool.tile([P, 512], R, tag="w")
                nc.sync.dma_start(out=wt, in_=wk_d[e * 128:(e + 1) * 128, :])
                for g in range(4):
                    nc.tensor.matmul(
                        pss[g], wt[:, g * 128:(g + 1) * 128], kt,
                        start=(e == 0), stop=(e == nE - 1), skip_group_check=True,
                    )
            for g in range(4):
                nc.vector.tensor_copy(KT[:, g, cs * 512:(cs + 1) * 512], pss[g])

        # ---- phase 4: Vproj -> Vn [P(skv), nKV, KV] ----
        Vn = bigv.tile([P, nKV, KV], R, tag="vn")
        for mq in range(4):
            pss = [psmm.tile([P, KV], F32, tag="mm", name=f"ps{_i}") for _i in range(4)]
            for e in range(nE):
                vt = apool.tile([P, 512], R, tag="a")
                nc.sync.dma_start(
                    out=vt, in_=vt_d[e * 128:(e + 1) * 128, mq * 512:(mq + 1) * 512]
                )
                wt = wpool.tile([P, 512], R, tag="w")
                nc.sync.dma_start(out=wt, in_=wv_d[e * 128:(e + 1) * 128, :])
                for j in range(4):
                    nc.tensor.matmul(
                        pss[j], vt[:, j * 128:(j + 1) * 128], wt,
                        start=(e == 0), stop=(e == nE - 1), skip_group_check=True,
                    )
            for j in range(4):
                nc.vector.tensor_copy(Vn[:, mq * 4 + j, :], pss[j])

        # ---- phase 5: attention ----
        OT = bigqo.tile([P, H, SQ], R, tag="qo")  # reuses qT slot
        for h in range(H):
            g = h // 4
            ps_rs = psra.tile([1, SQ], F32, tag="ra")
            ps_av = psra.tile([P, SQ], F32, tag="ra")
            PTh = [None, None]

            def rs_av(c):
                nc.tensor.matmul(
                    ps_rs, ones_col, PTh[c // 8][:, c % 8, :],
                    start=(c == 0), stop=(c == nKV - 1), skip_group_check=True,
                )
                nc.tensor.matmul(
                    ps_av, Vn[:, c, g * 128:(g + 1) * 128], PTh[c // 8][:, c % 8, :],
                    start=(c == 0), stop=(c == nKV - 1), skip_group_check=True,
                )

            for c in range(nKV):
                if c % 8 == 0:
                    PTh[c // 8] = ptp.tile([P, 8, SQ], R, tag="pt", name="PTh")
                ps_s = psmm.tile([P, SQ], F32, tag="mm")
                nc.tensor.matmul(
                    ps_s, KT[:, g, c * 128:(c + 1) * 128], QT[:, h, :],
                    start=True, stop=True,
                )
                nc.scalar.activation(
                    PTh[c // 8][:, c % 8, :], ps_s, AF.Exp,
                    bias=bias_sb[:, c:c + 1], scale=SC,
                )
                if c >= 1:
                    rs_av(c - 1)  # one-chunk skew: never wait on this chunk's exp
            rs_av(nKV - 1)

            rs_sb = small.tile([1, SQ], F32, tag="rs_sb")
            nc.vector.tensor_copy(rs_sb, ps_rs)
            bc_ps = psra.tile([P, SQ], F32, tag="ra", name="bc_ps")
            # plain-f32 rank-1 matmul: exact broadcast of the softmax denominator
            nc.tensor.matmul(bc_ps, ones_row, rs_sb, start=True, stop=True)
            recip_bc = small.tile([P, SQ], F32, tag="recip_bc")
            nc.vector.reciprocal_approx_fast(out=recip_bc, in_=bc_ps)
            nc.vector.tensor_mul(OT[:, h, :], ps_av, recip_bc)

        # ---- phase 6: Oproj -> yT ----
        for mq in range(4):
            pss = [psmm.tile([P, SQ], F32, tag="mm", name=f"ps{_i}") for _i in range(4)]
            for o in range(nE):
                wt = wpool.tile([P, 512], R, tag="w")
                nc.sync.dma_start(
                    out=wt, in_=wo_d[o * 128:(o + 1) * 128, mq * 512:(mq + 1) * 512]
                )
                for j in range(4):
                    nc.tensor.matmul(
                        pss[j], wt[:, j * 128:(j + 1) * 128], OT[:, o, :],
                        start=(o == 0), stop=(o == nE - 1), skip_group_check=True,
                    )
            for j in range(4):
                ys = ystg.tile([P, 512], F32, tag="y")
                nc.vector.tensor_copy(ys, pss[j])
                nc.sync.dma_start(
                    out=yt_d[(mq * 4 + j) * 128:(mq * 4 + j + 1) * 128, :], in_=ys
                )

    nc.compile()
    return nc


_nc = None


def _get_nc():
    global _nc
    if _nc is None:
        _nc = build_nc()
    return _nc


def _make_in_maps(query, key, value, mask, Wq, Wk, Wv, Wo):
    asf = lambda x: np.ascontiguousarray(x, dtype=np.float32)
    wq_r, wk_r, wv_r, wo_r = (asf(w) for w in (Wq, Wk, Wv, Wo))
    kts = [asf(np.asarray(key[b], np.float32).T) for b in range(B)]
    vts = [asf(np.asarray(value[b], np.float32).T) for b in range(B)]
    ms = [asf(mask[b]) for b in range(B)]
    in_maps = []
    for c in range(N_CORES):
        b, q0 = c // 4, (c % 4) * SQ
        in_maps.append({
            "qt": asf(np.asarray(query[b, q0:q0 + SQ], np.float32).T),
            "kt": kts[b],
            "vt": vts[b],
            "m": ms[b],
            "wq": wq_r, "wk": wk_r, "wv": wv_r, "wo": wo_r,
        })
    return in_maps


def run(query, key, value, mask, Wq, Wk, Wv, Wo, trace=False, trace_kwargs=None):
    from concourse.bass_utils import run_bass_kernel_spmd

    nc = _get_nc()
    in_maps = _make_in_maps(query, key, value, mask, Wq, Wk, Wv, Wo)
    res = run_bass_kernel_spmd(
        nc, in_maps, list(range(N_CORES)), trace=trace, **(trace_kwargs or {})
    )
    out = np.empty((B, S, E), np.float32)
    for c in range(N_CORES):
        b, q0 = c // 4, (c % 4) * SQ
        out[b, q0:q0 + SQ] = res.results[c]["yt"].T
    return out, res


def kernel(query, key, value, mask, Wq, Wk, Wv, Wo):
    out, _ = run(query, key, value, mask, Wq, Wk, Wv, Wo, trace=False)
    return out


# revision 5
# speedup vs baseline: 1.0523x; 1.0523x over previous
"""GQA attention kernel for 8 Trainium2 NeuronCores.

Sharding: sequence-parallel. Core c handles batch b = c//4 and query rows
[(c%4)*512, (c%4+1)*512) of that batch. Each core computes the full K/V
projection for its batch (duplicated 4x) so there are no collectives; the
host just concatenates the 8 output row-blocks.

All activations are kept feature-major on-chip. The host pre-transposes
q/k/v (and un-transposes the output), so the kernel contains NO PE
transposes -- every TensorE instruction is a productive matmul.

All matmul operands are fp16 (inputs converted on the host): fp16 streams
at 1 cycle/row (vs 1.5 for fp32r), enables fast weight loads, and halves
DMA traffic. PSUM accumulation stays fp32. Measured end-to-end relative
error ~5e-3 vs the fp32 reference.

  qT  [e, sq]   <- DMA (host-transposed fp16)
  QT  [d,h,sq]  <- Wq.T @ qT        (per 512-col quarter)
  KT  [d,g,skv] <- Wk.T @ kT        (per 512-row skv chunk; Wk resident)
  Vn  [skv,kv]  <- vT.T @ Wv        (Wv resident; mask folded into Vn rows)
  per head h (group g = h//4), per PAIR of skv chunks (2x128 rows):
    scoresT pair -> one [128,1024] PSUM pair        (2 matmuls)
    PT = exp(scoresT*scale)                         (one ACT op per pair)
    rowsum += maskcol.T @ PT  ;  OT[h] += Vn.T @ PT (PSUM acc, 1-pair skew)
  OT[h] *= 1/rowsum (broadcast via exact fp32 rank-1 matmul)
  YT = Wo.T @ OT -> DMA out (fp32), host un-transposes.

The mask is applied by zeroing rows of Vn and using the mask itself as
the rowsum stationary vector (exp(-1e9)=0 equivalence), so the exp needs
no per-chunk bias and pairs of chunks share one ACT instruction.
"""

import os
import sys

sys.path.insert(0, "/opt/trn_rl_repo")
if os.environ.get("JAX_PLATFORMS") == "cpu":
    del os.environ["JAX_PLATFORMS"]
os.environ.setdefault("MYCRO_LOCAL_CACHE", "1")

from contextlib import ExitStack

import numpy as np

import concourse.bass as bass
import concourse.bacc as bacc
import concourse.mybir as mybir
import concourse.tile as tile

P = 128
E = 2048          # embed dim
SQ = 512          # query rows per core
SKV = 2048        # kv sequence length
KV = 512          # kv projection width (4 kv heads * 128)
H = 16            # query heads
nE = E // P       # 16
nKV = SKV // P    # 16
SC = 1.0 / float(128.0) ** 0.5
B, S = 2, 2048
N_CORES = 8

F32 = mybir.dt.float32
F16 = mybir.dt.float16
AF = mybir.ActivationFunctionType


def build_nc():
    nc = bacc.Bacc(target_bir_lowering=False)

    qt_d = nc.dram_tensor("qt", [E, SQ], F16, kind="ExternalInput")
    kt_d = nc.dram_tensor("kt", [E, SKV], F16, kind="ExternalInput")
    vt_d = nc.dram_tensor("vt", [E, SKV], F16, kind="ExternalInput")
    m_d = nc.dram_tensor("m", [SKV], F32, kind="ExternalInput")
    wq_d = nc.dram_tensor("wq", [E, E], F16, kind="ExternalInput")
    wk_d = nc.dram_tensor("wk", [E, KV], F16, kind="ExternalInput")
    wv_d = nc.dram_tensor("wv", [E, KV], F16, kind="ExternalInput")
    wo_d = nc.dram_tensor("wo", [E, E], F16, kind="ExternalInput")
    yt_d = nc.dram_tensor("yt", [E, SQ], F32, kind="ExternalOutput")

    with ExitStack() as ctx:
        tc = ctx.enter_context(tile.TileContext(nc))
        consts = ctx.enter_context(tc.tile_pool(name="consts", bufs=1))
        wpool = ctx.enter_context(tc.tile_pool(name="wpool", bufs=4))
        apool = ctx.enter_context(tc.tile_pool(name="apool", bufs=4))
        bigq = ctx.enter_context(tc.tile_pool(name="bigq", bufs=1))
        bigk = ctx.enter_context(tc.tile_pool(name="bigk", bufs=1))
        bigv = ctx.enter_context(tc.tile_pool(name="bigv", bufs=1))
        bigqo = ctx.enter_context(tc.tile_pool(name="bigqo", bufs=1))
        wkres = ctx.enter_context(tc.tile_pool(name="wkres", bufs=1))
        wvres = ctx.enter_context(tc.tile_pool(name="wvres", bufs=1))
        ptp = ctx.enter_context(tc.tile_pool(name="ptp", bufs=2))
        small = ctx.enter_context(tc.tile_pool(name="small", bufs=2))
        psmm = ctx.enter_context(tc.tile_pool(name="psmm", bufs=3, space="PSUM"))
        psra = ctx.enter_context(tc.tile_pool(name="psra", bufs=2, space="PSUM"))
        ystg = ctx.enter_context(tc.tile_pool(name="ystg", bufs=4))

        # ---- constants ----
        ones_row = consts.tile([1, P], F32, tag="ones_r")
        nc.vector.memset(ones_row, 1.0)
        mask_sb = consts.tile([P, nKV], F32, tag="msk")
        nc.sync.dma_start(out=mask_sb, in_=m_d.rearrange("(a b) -> b a", b=P))
        # fp16 mask column: stationary vector for the rowsum matmuls, so
        # masked skv positions contribute 0 to the softmax denominator.
        mcol = consts.tile([P, nKV], F16, tag="mcol")
        nc.vector.tensor_copy(mcol, mask_sb)

        # ---- resident loads (issued up front, consumed later) ----
        qT = bigqo.tile([P, nE, SQ], F16, tag="qo")
        for e in range(nE):
            nc.sync.dma_start(out=qT[:, e, :], in_=qt_d[e * 128:(e + 1) * 128, :])
        wkr = wkres.tile([P, nE, KV], F16, tag="wk")
        nc.sync.dma_start(out=wkr, in_=wk_d.rearrange("(a b) c -> b a c", b=P))
        wvr = wvres.tile([P, nE, KV], F16, tag="wv")
        nc.sync.dma_start(out=wvr, in_=wv_d.rearrange("(a b) c -> b a c", b=P))

        def quad_psum():
            # two [P,2,512] pair-tiles = 4 bank-aligned fp32 accumulators
            prs = [psmm.tile([P, 2, 512], F32, tag="mm", name=f"pr{_i}")
                   for _i in range(2)]
            return prs, [prs[_j // 2][:, _j % 2, :] for _j in range(4)]

        # ---- phase 2: Qproj -> QT [P(d), H, SQ] ----
        QT = bigq.tile([P, H, SQ], F16, tag="qt")
        for mq in range(4):
            _, pss = quad_psum()
            for e in range(nE):
                wt = wpool.tile([P, 512], F16, tag="w")
                nc.sync.dma_start(
                    out=wt, in_=wq_d[e * 128:(e + 1) * 128, mq * 512:(mq + 1) * 512]
                )
                for j in range(4):
                    nc.tensor.matmul(
                        pss[j], wt[:, j * 128:(j + 1) * 128], qT[:, e, :],
                        start=(e == 0), stop=(e == nE - 1), skip_group_check=True,
                    )
            for j in range(4):
                nc.vector.tensor_copy(QT[:, mq * 4 + j, :], pss[j])

        # ---- phase 3: Kproj -> KT [P(d), G, SKV] ----
        KT = bigk.tile([P, 4, SKV], F16, tag="kt")
        for cs in range(4):
            _, pss = quad_psum()
            for e in range(nE):
                kt = apool.tile([P, 512], F16, tag="a")
                nc.sync.dma_start(
                    out=kt, in_=kt_d[e * 128:(e + 1) * 128, cs * 512:(cs + 1) * 512]
                )
                for g in range(4):
                    nc.tensor.matmul(
                        pss[g], wkr[:, e, g * 128:(g + 1) * 128], kt,
                        start=(e == 0), stop=(e == nE - 1), skip_group_check=True,
                    )
            for g in range(4):
                nc.vector.tensor_copy(KT[:, g, cs * 512:(cs + 1) * 512], pss[g])

        # ---- phase 4: Vproj -> Vn [P(skv), nKV, KV], mask folded in ----
        Vn = bigv.tile([P, nKV, KV], F16, tag="vn")
        for mq in range(4):
            _, pss = quad_psum()
            for e in range(nE):
                vt = apool.tile([P, 512], F16, tag="a")
                nc.sync.dma_start(
                    out=vt, in_=vt_d[e * 128:(e + 1) * 128, mq * 512:(mq + 1) * 512]
                )
                for j in range(4):
                    nc.tensor.matmul(
                        pss[j], vt[:, j * 128:(j + 1) * 128], wvr[:, e, :],
                        start=(e == 0), stop=(e == nE - 1), skip_group_check=True,
                    )
            for j in range(4):
                c = mq * 4 + j
                # rows of V for masked skv positions are zeroed here
                nc.vector.tensor_scalar_mul(Vn[:, c, :], pss[j], mask_sb[:, c:c + 1])

        # ---- phase 5: attention ----
        OT = bigqo.tile([P, H, SQ], F16, tag="qo")  # reuses qT slot
        for h in range(H):
            g = h // 4
            ps_rs = psra.tile([1, SQ], F32, tag="ra")
            ps_av = psra.tile([P, SQ], F32, tag="ra")
            PTh = [None, None]

            def rs_av(c):
                pt_c = PTh[c // 8][:, c % 8, :]
                nc.tensor.matmul(
                    ps_rs, mcol[:, c:c + 1], pt_c,
                    start=(c == 0), stop=(c == nKV - 1), skip_group_check=True,
                )
                nc.tensor.matmul(
                    ps_av, Vn[:, c, g * 128:(g + 1) * 128], pt_c,
                    start=(c == 0), stop=(c == nKV - 1), skip_group_check=True,
                )

            for p in range(nKV // 2):  # pairs of skv chunks
                c0 = 2 * p
                if c0 % 8 == 0:
                    PTh[c0 // 8] = ptp.tile([P, 8, SQ], F16, tag="pt", name="PTh")
                ps_s = psmm.tile([P, 2, 512], F32, tag="mm", name="ps_s")
                for i in range(2):
                    nc.tensor.matmul(
                        ps_s[:, i, :],
                        KT[:, g, (c0 + i) * 128:(c0 + i + 1) * 128], QT[:, h, :],
                        start=True, stop=True,
                    )
                # one 1024-wide exp per pair (no bias needed: mask is folded
                # into mcol/Vn)
                nc.scalar.activation(
                    PTh[c0 // 8][:, c0 % 8:c0 % 8 + 2, :], ps_s, AF.Exp, scale=SC,
                )
                if p >= 1:
                    rs_av(c0 - 2)
                    rs_av(c0 - 1)
            rs_av(nKV - 2)
            rs_av(nKV - 1)

            rs_sb = small.tile([1, SQ], F32, tag="rs_sb")
            nc.vector.tensor_copy(rs_sb, ps_rs)
            bc_ps = psra.tile([P, SQ], F32, tag="ra", name="bc_ps")
            # plain-f32 rank-1 matmul: exact broadcast of the softmax denominator
            nc.tensor.matmul(bc_ps, ones_row, rs_sb, start=True, stop=True)
            recip_bc = small.tile([P, SQ], F32, tag="recip_bc")
            nc.vector.reciprocal_approx_fast(out=recip_bc, in_=bc_ps)
            nc.vector.tensor_mul(OT[:, h, :], ps_av, recip_bc)

        # ---- phase 6: Oproj -> yT ----
        for mq in range(4):
            _, pss = quad_psum()
            for o in range(nE):
                wt = wpool.tile([P, 512], F16, tag="w")
                nc.sync.dma_start(
                    out=wt, in_=wo_d[o * 128:(o + 1) * 128, mq * 512:(mq + 1) * 512]
                )
                for j in range(4):
                    nc.tensor.matmul(
                        pss[j], wt[:, j * 128:(j + 1) * 128], OT[:, o, :],
                        start=(o == 0), stop=(o == nE - 1), skip_group_check=True,
                    )
            for j in range(4):
                ys = ystg.tile([P, 512], F32, tag="y")
                nc.vector.tensor_copy(ys, pss[j])
                nc.sync.dma_start(
                    out=yt_d[(mq * 4 + j) * 128:(mq * 4 + j + 1) * 128, :], in_=ys
                )

    nc.compile()
    return nc


_nc = None


def _get_nc():
    global _nc
    if _nc is None:
        _nc = build_nc()
    return _nc


def _make_in_maps(query, key, value, mask, Wq, Wk, Wv, Wo):
    f16 = lambda x: np.ascontiguousarray(np.asarray(x, np.float32), dtype=np.float16)
    wq_h, wk_h, wv_h, wo_h = (f16(w) for w in (Wq, Wk, Wv, Wo))
    kts = [f16(np.asarray(key[b], np.float32).T) for b in range(B)]
    vts = [f16(np.asarray(value[b], np.float32).T) for b in range(B)]
    ms = [np.ascontiguousarray(mask[b], dtype=np.float32) for b in range(B)]
    in_maps = []
    for c in range(N_CORES):
        b, q0 = c // 4, (c % 4) * SQ
        in_maps.append({
            "qt": f16(np.asarray(query[b, q0:q0 + SQ], np.float32).T),
            "kt": kts[b],
            "vt": vts[b],
            "m": ms[b],
            "wq": wq_h, "wk": wk_h, "wv": wv_h, "wo": wo_h,
        })
    return in_maps


def run(query, key, value, mask, Wq, Wk, Wv, Wo, trace=False, trace_kwargs=None):
    from concourse.bass_utils import run_bass_kernel_spmd

    nc = _get_nc()
    in_maps = _make_in_maps(query, key, value, mask, Wq, Wk, Wv, Wo)
    res = run_bass_kernel_spmd(
        nc, in_maps, list(range(N_CORES)), trace=trace, **(trace_kwargs or {})
    )
    out = np.empty((B, S, E), np.float32)
    for c in range(N_CORES):
        b, q0 = c // 4, (c % 4) * SQ
        out[b, q0:q0 + SQ] = res.results[c]["yt"].T
    return out, res


def kernel(query, key, value, mask, Wq, Wk, Wv, Wo):
    out, _ = run(query, key, value, mask, Wq, Wk, Wv, Wo, trace=False)
    return out


# revision 9
# speedup vs baseline: 1.4846x; 1.4108x over previous
"""GQA attention kernel for 8 Trainium2 NeuronCores.

Sharding: tensor-parallel over kv-head groups x data-parallel over batch.
Core c handles batch b = c//4 and kv-head group g = c%4 (query heads
4g..4g+3) for ALL 2048 query positions of its batch. Wq/Wk/Wv are split
column-wise by head group, Wo row-wise; each core emits a partial output
projection and the host sums the 4 partials per batch (the "all-reduce
after output projection" of classic TP, done on the host).

vs. pure sequence-parallel this removes the 4x-duplicated K/V projection
(the only duplicated compute): per-core matmul count drops from 1792 to
1600, of which 256 (Vproj) stream only 128 columns.

All activations are kept feature-major on-chip. The host pre-transposes
q/k/v (and un-transposes + reduces the output), so the kernel contains
NO PE transposes. All matmul operands are fp16 (1 cycle/row streaming,
half DMA); PSUM accumulation stays fp32. Relative error ~5e-3.

  qT  [e, s]    <- DMA slabs (host-transposed fp16), streamed per s4
  QT  [d,4h,s]  <- Wq_g.T @ qT      (Wq_g resident, 16KB/partition)
  KT  [d, skv]  <- Wk_g.T @ kT      (one kv head; Wk_g resident)
  Vn  [skv,128] <- vT.T @ Wv_g      (mask folded into Vn rows)
  per (head h, query chunk s4), per PAIR of skv chunks:
    scoresT pair -> one [128,1024] PSUM pair        (2 matmuls)
    PT = exp(scoresT*scale)                         (one ACT op per pair)
    rowsum += maskcol.T @ PT ; OT[h] += Vn.T @ PT   (PSUM acc, 1-pair skew)
  OT *= 1/rowsum (exact fp32 rank-1 broadcast matmul, emitted lazily so
    the PE never stalls at a chunk boundary)
  yT_partial = Wo_g.T @ OT -> DMA out (fp16), host sums partials.
"""

import os
import sys

sys.path.insert(0, "/opt/trn_rl_repo")
if os.environ.get("JAX_PLATFORMS") == "cpu":
    del os.environ["JAX_PLATFORMS"]
os.environ.setdefault("MYCRO_LOCAL_CACHE", "1")

from contextlib import ExitStack

import numpy as np

import concourse.bass as bass
import concourse.bacc as bacc
import concourse.mybir as mybir
import concourse.tile as tile

P = 128
E = 2048          # embed dim
S = 2048          # sequence length (queries and kv)
GQ = 512          # per-group query-projection width (4 heads * 128)
GK = 128          # per-group kv width (1 kv head)
nE = E // P       # 16
nKV = S // P      # 16
SC = 1.0 / float(128.0) ** 0.5
B = 2
N_CORES = 8

F32 = mybir.dt.float32
F16 = mybir.dt.float16
AF = mybir.ActivationFunctionType


def build_nc():
    nc = bacc.Bacc(target_bir_lowering=False)

    qt_d = nc.dram_tensor("qt", [E, S], F16, kind="ExternalInput")
    kt_d = nc.dram_tensor("kt", [E, S], F16, kind="ExternalInput")
    vt_d = nc.dram_tensor("vt", [E, S], F16, kind="ExternalInput")
    m_d = nc.dram_tensor("m", [S], F32, kind="ExternalInput")
    wq_d = nc.dram_tensor("wq", [E, GQ], F16, kind="ExternalInput")
    wk_d = nc.dram_tensor("wk", [E, GK], F16, kind="ExternalInput")
    wv_d = nc.dram_tensor("wv", [E, GK], F16, kind="ExternalInput")
    wo_d = nc.dram_tensor("wo", [GQ, E], F16, kind="ExternalInput")
    yt_d = nc.dram_tensor("yt", [E, S], F16, kind="ExternalOutput")

    with ExitStack() as ctx:
        tc = ctx.enter_context(tile.TileContext(nc))
        consts = ctx.enter_context(tc.tile_pool(name="consts", bufs=1))
        wqres = ctx.enter_context(tc.tile_pool(name="wqres", bufs=1))
        wkres = ctx.enter_context(tc.tile_pool(name="wkres", bufs=1))
        wvres = ctx.enter_context(tc.tile_pool(name="wvres", bufs=1))
        wores = ctx.enter_context(tc.tile_pool(name="wores", bufs=1))
        qslab = ctx.enter_context(tc.tile_pool(name="qslab", bufs=2))
        kvslab = ctx.enter_context(tc.tile_pool(name="kvslab", bufs=2))
        bigq = ctx.enter_context(tc.tile_pool(name="bigq", bufs=1))
        bigk = ctx.enter_context(tc.tile_pool(name="bigk", bufs=1))
        bigv = ctx.enter_context(tc.tile_pool(name="bigv", bufs=1))
        bigo = ctx.enter_context(tc.tile_pool(name="bigo", bufs=1))
        ptp = ctx.enter_context(tc.tile_pool(name="ptp", bufs=2))
        small = ctx.enter_context(tc.tile_pool(name="small", bufs=2))
        psmm = ctx.enter_context(tc.tile_pool(name="psmm", bufs=2, space="PSUM"))
        psra = ctx.enter_context(tc.tile_pool(name="psra", bufs=2, space="PSUM"))
        psbc = ctx.enter_context(tc.tile_pool(name="psbc", bufs=1, space="PSUM"))
        ystg = ctx.enter_context(tc.tile_pool(name="ystg", bufs=4))

        # ---- constants ----
        ones_row = consts.tile([1, P], F32, tag="ones_r")
        nc.vector.memset(ones_row, 1.0)
        mask_sb = consts.tile([P, nKV], F32, tag="msk")
        nc.sync.dma_start(out=mask_sb, in_=m_d.rearrange("(a b) -> b a", b=P))
        # fp16 mask column: stationary vector for the rowsum matmuls, so
        # masked skv positions contribute 0 to the softmax denominator.
        mcol = consts.tile([P, nKV], F16, tag="mcol")
        nc.vector.tensor_copy(mcol, mask_sb)

        # ---- resident weights (wq first: Qproj starts immediately) ----
        wqr = wqres.tile([P, nE, GQ], F16, tag="wq")
        for e in range(nE):
            nc.sync.dma_start(out=wqr[:, e, :], in_=wq_d[e * 128:(e + 1) * 128, :])
        wkr = wkres.tile([P, nE, GK], F16, tag="wk")
        nc.sync.dma_start(out=wkr, in_=wk_d.rearrange("(a b) c -> b a c", b=P))
        wvr = wvres.tile([P, nE, GK], F16, tag="wv")
        nc.sync.dma_start(out=wvr, in_=wv_d.rearrange("(a b) c -> b a c", b=P))

        def load_slab(pool, src_d, col0, tag):
            # [P, nE, 512] fp16 slab of a transposed activation, as 4
            # sub-DMAs so consumers wake up 4 e-tiles at a time
            sl = pool.tile([P, nE, 512], F16, tag=tag, name=f"slab_{tag}")
            for q in range(4):
                nc.sync.dma_start(
                    out=sl[:, q * 4:(q + 1) * 4, :],
                    in_=src_d[:, col0:col0 + 512].rearrange(
                        "(a b) c -> b a c", b=P
                    )[:, q * 4:(q + 1) * 4, :],
                )
            return sl

        def quad_psum():
            prs = [psmm.tile([P, 2, 512], F32, tag="mm", name=f"pr{_i}")
                   for _i in range(2)]
            return [prs[_j // 2][:, _j % 2, :] for _j in range(4)]

        # ---- phase 1: Qproj -> QT [P(d), 4, S] ----
        QT = bigq.tile([P, 4, S], F16, tag="qt")
        for s4 in range(4):
            qsl = load_slab(qslab, qt_d, s4 * 512, "q")
            pss = quad_psum()
            for e in range(nE):
                for hc in range(4):
                    nc.tensor.matmul(
                        pss[hc], wqr[:, e, hc * 128:(hc + 1) * 128], qsl[:, e, :],
                        start=(e == 0), stop=(e == nE - 1), skip_group_check=True,
                    )
            for hc in range(4):
                nc.vector.tensor_copy(QT[:, hc, s4 * 512:(s4 + 1) * 512], pss[hc])

        # ---- phase 2: Kproj -> KT [P(d), S] ----
        KT = bigk.tile([P, S], F16, tag="kt")
        for cs in range(2):  # two 1024-wide column groups, one PSUM pair each
            ksl0 = load_slab(kvslab, kt_d, (2 * cs) * 512, "kv")
            ksl1 = load_slab(kvslab, kt_d, (2 * cs + 1) * 512, "kv")
            pr = psmm.tile([P, 2, 512], F32, tag="mm", name="prk")
            for e in range(nE):
                for i, ksl in enumerate((ksl0, ksl1)):
                    nc.tensor.matmul(
                        pr[:, i, :], wkr[:, e, :], ksl[:, e, :],
                        start=(e == 0), stop=(e == nE - 1), skip_group_check=True,
                    )
            nc.vector.tensor_copy(
                KT[:, (2 * cs) * 512:(2 * cs + 2) * 512],
                pr.rearrange("p a b -> p (a b)"),
            )

        # ---- phase 3: Vproj -> Vn [P(skv), nKV, GK], mask folded in ----
        Vn = bigv.tile([P, nKV, GK], F16, tag="vn")
        for mq in range(4):
            vsl = load_slab(kvslab, vt_d, mq * 512, "kv")
            pss = quad_psum()
            for e in range(nE):
                for j in range(4):
                    nc.tensor.matmul(
                        pss[j][:, 0:GK], vsl[:, e, j * 128:(j + 1) * 128],
                        wvr[:, e, :],
                        start=(e == 0), stop=(e == nE - 1), skip_group_check=True,
                    )
            for j in range(4):
                c = mq * 4 + j
                # rows of V for masked skv positions are zeroed here
                nc.vector.tensor_scalar_mul(
                    Vn[:, c, :], pss[j][:, 0:GK], mask_sb[:, c:c + 1]
                )

        # resident Wo: needed only in phase 5, DMA hides under attention
        wor = wores.tile([P, 4, E], F16, tag="wo")
        nc.sync.dma_start(out=wor, in_=wo_d.rearrange("(a b) c -> b a c", b=P))

        # ---- phase 4: attention ----
        OT = bigo.tile([P, 4, S], F16, tag="ot")
        pending = None  # lazy epilogue: (ps_rs, ps_av, h, s4)

        def flush_epilogue():
            nonlocal pending
            if pending is None:
                return
            ps_rs, ps_av, h, s4 = pending
            pending = None
            rs_sb = small.tile([1, 512], F32, tag="rs_sb")
            nc.vector.tensor_copy(rs_sb, ps_rs)
            bc_ps = psbc.tile([P, 512], F32, tag="bc", name="bc_ps")
            # plain-f32 rank-1 matmul: exact broadcast of the denominator
            nc.tensor.matmul(bc_ps, ones_row, rs_sb, start=True, stop=True)
            recip_bc = small.tile([P, 512], F32, tag="recip_bc")
            nc.vector.reciprocal_approx_fast(out=recip_bc, in_=bc_ps)
            nc.vector.tensor_mul(
                OT[:, h, s4 * 512:(s4 + 1) * 512], ps_av, recip_bc
            )

        for h in range(4):
            for s4 in range(4):
                qs = QT[:, h, s4 * 512:(s4 + 1) * 512]
                ps_rs = psra.tile([1, 512], F32, tag="ra")
                ps_av = psra.tile([P, 512], F32, tag="ra")
                PTh = [None, None]

                def rs_av(c):
                    pt_c = PTh[c // 8][:, c % 8, :]
                    nc.tensor.matmul(
                        ps_rs, mcol[:, c:c + 1], pt_c,
                        start=(c == 0), stop=(c == nKV - 1), skip_group_check=True,
                    )
                    nc.tensor.matmul(
                        ps_av, Vn[:, c, :], pt_c,
                        start=(c == 0), stop=(c == nKV - 1), skip_group_check=True,
                    )

                for p in range(nKV // 2):  # pairs of skv chunks
                    c0 = 2 * p
                    if c0 % 8 == 0:
                        PTh[c0 // 8] = ptp.tile([P, 8, 512], F16, tag="pt",
                                                name="PTh")
                    ps_s = psmm.tile([P, 2, 512], F32, tag="mm", name="ps_s")
                    for i in range(2):
                        nc.tensor.matmul(
                            ps_s[:, i, :], KT[:, (c0 + i) * 128:(c0 + i + 1) * 128],
                            qs, start=True, stop=True,
                        )
                    if p == 1:
                        # previous (h,s4)'s epilogue lands here so its bc
                        # matmul never stalls the PE at the boundary
                        flush_epilogue()
                    nc.scalar.activation(
                        PTh[c0 // 8][:, c0 % 8:c0 % 8 + 2, :], ps_s, AF.Exp,
                        scale=SC,
                    )
                    if p >= 1:
                        rs_av(c0 - 2)
                        rs_av(c0 - 1)
                rs_av(nKV - 2)
                rs_av(nKV - 1)
                pending = (ps_rs, ps_av, h, s4)
        flush_epilogue()

        # ---- phase 5: Oproj -> yT partial ----
        for s4 in range(4):
            for q4 in range(4):
                pss = quad_psum()
                for o in range(4):
                    for j in range(4):
                        nc.tensor.matmul(
                            pss[j],
                            wor[:, o, (q4 * 4 + j) * 128:(q4 * 4 + j + 1) * 128],
                            OT[:, o, s4 * 512:(s4 + 1) * 512],
                            start=(o == 0), stop=(o == 3), skip_group_check=True,
                        )
                for j in range(4):
                    ys = ystg.tile([P, 512], F16, tag="y")
                    nc.vector.tensor_copy(ys, pss[j])
                    nc.sync.dma_start(
                        out=yt_d[(q4 * 4 + j) * 128:(q4 * 4 + j + 1) * 128,
                                 s4 * 512:(s4 + 1) * 512],
                        in_=ys,
                    )

    nc.compile()
    return nc


_nc = None


def _get_nc():
    global _nc
    if _nc is None:
        _nc = build_nc()
    return _nc


def _make_in_maps(query, key, value, mask, Wq, Wk, Wv, Wo):
    f16 = lambda x: np.ascontiguousarray(np.asarray(x, np.float32), dtype=np.float16)
    qts = [f16(np.asarray(query[b], np.float32).T) for b in range(B)]
    kts = [f16(np.asarray(key[b], np.float32).T) for b in range(B)]
    vts = [f16(np.asarray(value[b], np.float32).T) for b in range(B)]
    ms = [np.ascontiguousarray(mask[b], dtype=np.float32) for b in range(B)]
    wq_h, wk_h, wv_h, wo_h = (f16(w) for w in (Wq, Wk, Wv, Wo))
    in_maps = []
    for c in range(N_CORES):
        b, g = c // 4, c % 4
        in_maps.append({
            "qt": qts[b],
            "kt": kts[b],
            "vt": vts[b],
            "m": ms[b],
            "wq": f16(wq_h[:, g * GQ:(g + 1) * GQ]),
            "wk": f16(wk_h[:, g * GK:(g + 1) * GK]),
            "wv": f16(wv_h[:, g * GK:(g + 1) * GK]),
            "wo": f16(wo_h[g * GQ:(g + 1) * GQ, :]),
        })
    return in_maps


def run(query, key, value, mask, Wq, Wk, Wv, Wo, trace=False, trace_kwargs=None):
    from concourse.bass_utils import run_bass_kernel_spmd

    nc = _get_nc()
    in_maps = _make_in_maps(query, key, value, mask, Wq, Wk, Wv, Wo)
    res = run_bass_kernel_spmd(
        nc, in_maps, list(range(N_CORES)), trace=trace, **(trace_kwargs or {})
    )
    out = np.empty((B, S, E), np.float32)
    for b in range(B):
        acc = np.zeros((E, S), np.float32)
        for g in range(4):
            acc += res.results[b * 4 + g]["yt"].astype(np.float32)
        out[b] = acc.T
    return out, res


def kernel(query, key, value, mask, Wq, Wk, Wv, Wo):
    out, _ = run(query, key, value, mask, Wq, Wk, Wv, Wo, trace=False)
    return out


# revision 11
# speedup vs baseline: 1.5893x; 1.0705x over previous
"""GQA attention kernel for 8 Trainium2 NeuronCores.

Sharding: tensor-parallel over kv-head groups x data-parallel over batch.
Core c handles batch b = c//4 and kv-head group g = c%4 (query heads
4g..4g+3) for ALL 2048 query positions of its batch. Wq/Wk/Wv are split
column-wise by head group, Wo row-wise; each core emits a partial output
projection and the host sums the 4 partials per batch (the "all-reduce
after output projection" of classic TP, done on the host). This removes
the K/V-projection duplication that pure sequence-parallel pays.

All activations are kept feature-major on-chip; the host pre-transposes
AND pre-packs every streamed tensor into its exact SBUF slab layout, so
each DMA line is >=4KB-contiguous (full DMA bandwidth) and the kernel
contains NO PE transposes. All matmul operands are fp16 (1 cycle/row
streaming at N=512 -> ~216ns/matmul, the PE floor); PSUM accumulation
stays fp32. The host un-transposes + reduces the output. Rel err ~5e-3.

Pipeline (emission order interleaves DMA-hungry Kproj blocks between
Qproj/Vproj blocks so the DMA engines never starve the PE):

  Q0 K0 Q1 K1 V0 Q2 K2 V1 Q3 K3 V2 V3   (projections, PSUM-quad blocks)
  for s4 (query 512-block): 4 heads of attention, then Oproj(s4)

  attention per (head, s4), per PAIR of skv chunks:
    scoresT pair -> one [128,1024] PSUM pair        (2 matmuls)
    PT = exp(scoresT*scale)                         (one ACT op per pair)
    rowsum += maskcol.T @ PT ; OT[h] += Vn.T @ PT   (PSUM acc, 1-pair skew)
  OT *= 1/rowsum via exact fp32 rank-1 broadcast matmul, emitted lazily
  one iteration later so the PE never stalls on the epilogue.

The mask is applied by zeroing rows of Vn and using the mask itself as
the rowsum stationary vector (exp(-1e9)=0 equivalence), so the exp needs
no per-chunk bias and pairs of chunks share one ACT instruction.
"""

import os
import sys

sys.path.insert(0, "/opt/trn_rl_repo")
if os.environ.get("JAX_PLATFORMS") == "cpu":
    del os.environ["JAX_PLATFORMS"]
os.environ.setdefault("MYCRO_LOCAL_CACHE", "1")

from contextlib import ExitStack

import numpy as np

import concourse.bass as bass
import concourse.bacc as bacc
import concourse.mybir as mybir
import concourse.tile as tile

P = 128
E = 2048          # embed dim
S = 2048          # sequence length (queries and kv)
GQ = 512          # per-group query-projection width (4 heads * 128)
GK = 128          # per-group kv width (1 kv head)
nE = E // P       # 16
nKV = S // P      # 16
SC = 1.0 / float(128.0) ** 0.5
B = 2
N_CORES = 8

F32 = mybir.dt.float32
F16 = mybir.dt.float16
AF = mybir.ActivationFunctionType


def build_nc():
    nc = bacc.Bacc(target_bir_lowering=False)

    # activations prepacked on the host as [s4][p][e][512] slabs
    qt_d = nc.dram_tensor("qt", [4, P, nE, 512], F16, kind="ExternalInput")
    kt_d = nc.dram_tensor("kt", [4, P, nE, 512], F16, kind="ExternalInput")
    vt_d = nc.dram_tensor("vt", [4, P, nE, 512], F16, kind="ExternalInput")
    m_d = nc.dram_tensor("m", [S], F32, kind="ExternalInput")
    # weights prepacked as [p][e][cols]
    wq_d = nc.dram_tensor("wq", [P, nE, GQ], F16, kind="ExternalInput")
    wk_d = nc.dram_tensor("wk", [P, nE, GK], F16, kind="ExternalInput")
    wv_d = nc.dram_tensor("wv", [P, nE, GK], F16, kind="ExternalInput")
    wo_d = nc.dram_tensor("wo", [P, 4, E], F16, kind="ExternalInput")
    yt_d = nc.dram_tensor("yt", [E, S], F16, kind="ExternalOutput")

    with ExitStack() as ctx:
        tc = ctx.enter_context(tile.TileContext(nc))
        consts = ctx.enter_context(tc.tile_pool(name="consts", bufs=1))
        wqres = ctx.enter_context(tc.tile_pool(name="wqres", bufs=1))
        wkres = ctx.enter_context(tc.tile_pool(name="wkres", bufs=1))
        wvres = ctx.enter_context(tc.tile_pool(name="wvres", bufs=1))
        wores = ctx.enter_context(tc.tile_pool(name="wores", bufs=1))
        qslab = ctx.enter_context(tc.tile_pool(name="qslab", bufs=2))
        kvslab = ctx.enter_context(tc.tile_pool(name="kvslab", bufs=2))
        bigq = ctx.enter_context(tc.tile_pool(name="bigq", bufs=1))
        bigk = ctx.enter_context(tc.tile_pool(name="bigk", bufs=1))
        bigv = ctx.enter_context(tc.tile_pool(name="bigv", bufs=1))
        bigo = ctx.enter_context(tc.tile_pool(name="bigo", bufs=1))
        ptp = ctx.enter_context(tc.tile_pool(name="ptp", bufs=2))
        small = ctx.enter_context(tc.tile_pool(name="small", bufs=2))
        psmm = ctx.enter_context(tc.tile_pool(name="psmm", bufs=2, space="PSUM"))
        psra = ctx.enter_context(tc.tile_pool(name="psra", bufs=2, space="PSUM"))
        psbc = ctx.enter_context(tc.tile_pool(name="psbc", bufs=1, space="PSUM"))
        ystg = ctx.enter_context(tc.tile_pool(name="ystg", bufs=4))

        # ---- constants ----
        ones_row = consts.tile([1, P], F32, tag="ones_r")
        nc.vector.memset(ones_row, 1.0)
        mask_sb = consts.tile([P, nKV], F32, tag="msk")
        nc.sync.dma_start(out=mask_sb, in_=m_d.rearrange("(a b) -> b a", b=P))
        mcol = consts.tile([P, nKV], F16, tag="mcol")
        nc.vector.tensor_copy(mcol, mask_sb)

        # ---- resident weights; wq interleaved with the first q slab so
        # ---- the PE starts streaming within ~3us of kernel start
        wqr = wqres.tile([P, nE, GQ], F16, tag="wq")
        QT = bigq.tile([P, 4, S], F16, tag="qt")
        qsl0 = qslab.tile([P, nE, 512], F16, tag="q", name="qsl0")
        for q in range(4):
            sl = slice(q * 4, (q + 1) * 4)
            nc.sync.dma_start(out=wqr[:, sl, :], in_=wq_d[:, sl, :])
            nc.sync.dma_start(out=qsl0[:, sl, :], in_=qt_d[0][:, sl, :])
        wkr = wkres.tile([P, nE, GK], F16, tag="wk")
        nc.sync.dma_start(out=wkr, in_=wk_d[:, :, :])
        wvr = wvres.tile([P, nE, GK], F16, tag="wv")
        nc.sync.dma_start(out=wvr, in_=wv_d[:, :, :])

        def load_slab(pool, src_d, s4, tag):
            sl = pool.tile([P, nE, 512], F16, tag=tag, name=f"slab_{tag}")
            for q in range(4):
                nc.sync.dma_start(
                    out=sl[:, q * 4:(q + 1) * 4, :],
                    in_=src_d[s4][:, q * 4:(q + 1) * 4, :],
                )
            return sl

        def quad_psum():
            prs = [psmm.tile([P, 2, 512], F32, tag="mm", name=f"pr{_i}")
                   for _i in range(2)]
            return [prs[_j // 2][:, _j % 2, :] for _j in range(4)]

        # ---- projection blocks ----
        KT = bigk.tile([P, S], F16, tag="kt")
        Vn = bigv.tile([P, nKV, GK], F16, tag="vn")

        def q_block(s4, qsl):
            pss = quad_psum()
            for e in range(nE):
                for hc in range(4):
                    nc.tensor.matmul(
                        pss[hc], wqr[:, e, hc * 128:(hc + 1) * 128], qsl[:, e, :],
                        start=(e == 0), stop=(e == nE - 1), skip_group_check=True,
                    )
            for hc in range(4):
                nc.vector.tensor_copy(QT[:, hc, s4 * 512:(s4 + 1) * 512], pss[hc])

        def k_block(cs):
            ksl = load_slab(kvslab, kt_d, cs, "kv")
            pr = psmm.tile([P, 2, 512], F32, tag="mm", name="prk")
            for e in range(nE):
                nc.tensor.matmul(
                    pr[:, 0, :], wkr[:, e, :], ksl[:, e, :],
                    start=(e == 0), stop=(e == nE - 1), skip_group_check=True,
                )
            nc.vector.tensor_copy(KT[:, cs * 512:(cs + 1) * 512], pr[:, 0, :])

        def v_block(mq):
            vsl = load_slab(kvslab, vt_d, mq, "kv")
            pss = quad_psum()
            for e in range(nE):
                for j in range(4):
                    nc.tensor.matmul(
                        pss[j][:, 0:GK], vsl[:, e, j * 128:(j + 1) * 128],
                        wvr[:, e, :],
                        start=(e == 0), stop=(e == nE - 1), skip_group_check=True,
                    )
            for j in range(4):
                c = mq * 4 + j
                # rows of V for masked skv positions are zeroed here
                nc.vector.tensor_scalar_mul(
                    Vn[:, c, :], pss[j][:, 0:GK], mask_sb[:, c:c + 1]
                )

        # interleave: K blocks are DMA-hungry (2MB per 3.5us of PE work),
        # so they sit between Q/V blocks and prefetch during them.
        q_block(0, qsl0)
        k_block(0)
        q_block(1, load_slab(qslab, qt_d, 1, "q"))
        k_block(1)
        v_block(0)
        q_block(2, load_slab(qslab, qt_d, 2, "q"))
        k_block(2)
        v_block(1)
        q_block(3, load_slab(qslab, qt_d, 3, "q"))
        k_block(3)
        v_block(2)
        v_block(3)

        # resident Wo: needed in Oproj, DMA hides under early attention
        wor = wores.tile([P, 4, E], F16, tag="wo")
        nc.sync.dma_start(out=wor, in_=wo_d[:, :, :])

        # ---- attention + output projection, query-block-major ----
        OT = bigo.tile([P, 4, S], F16, tag="ot")
        pending = None  # lazy epilogue: (ps_rs, ps_av, h, s4)

        def flush_epilogue():
            nonlocal pending
            if pending is None:
                return
            ps_rs, ps_av, h, s4 = pending
            pending = None
            rs_sb = small.tile([1, 512], F32, tag="rs_sb")
            nc.vector.tensor_copy(rs_sb, ps_rs)
            bc_ps = psbc.tile([P, 512], F32, tag="bc", name="bc_ps")
            # plain-f32 rank-1 matmul: exact broadcast of the denominator
            nc.tensor.matmul(bc_ps, ones_row, rs_sb, start=True, stop=True)
            recip_bc = small.tile([P, 512], F32, tag="recip_bc")
            nc.vector.reciprocal_approx_fast(out=recip_bc, in_=bc_ps)
            nc.vector.tensor_mul(
                OT[:, h, s4 * 512:(s4 + 1) * 512], ps_av, recip_bc
            )

        for s4 in range(4):
            for h in range(4):
                qs = QT[:, h, s4 * 512:(s4 + 1) * 512]
                ps_rs = psra.tile([1, 512], F32, tag="ra")
                ps_av = psra.tile([P, 512], F32, tag="ra")
                PTh = [None, None]

                def rs_av(c):
                    pt_c = PTh[c // 8][:, c % 8, :]
                    nc.tensor.matmul(
                        ps_rs, mcol[:, c:c + 1], pt_c,
                        start=(c == 0), stop=(c == nKV - 1),
                        skip_group_check=True,
                    )
                    nc.tensor.matmul(
                        ps_av, Vn[:, c, :], pt_c,
                        start=(c == 0), stop=(c == nKV - 1),
                        skip_group_check=True,
                    )

                for p in range(nKV // 2):  # pairs of skv chunks
                    c0 = 2 * p
                    if c0 % 8 == 0:
                        PTh[c0 // 8] = ptp.tile([P, 8, 512], F16, tag="pt",
                                                name="PTh")
                    ps_s = psmm.tile([P, 2, 512], F32, tag="mm", name="ps_s")
                    for i in range(2):
                        nc.tensor.matmul(
                            ps_s[:, i, :],
                            KT[:, (c0 + i) * 128:(c0 + i + 1) * 128],
                            qs, start=True, stop=True,
                        )
                    if p == 1:
                        # previous iteration's epilogue lands here so its
                        # bc matmul never stalls the PE at the boundary
                        flush_epilogue()
                    nc.scalar.activation(
                        PTh[c0 // 8][:, c0 % 8:c0 % 8 + 2, :], ps_s, AF.Exp,
                        scale=SC,
                    )
                    if p >= 1:
                        rs_av(c0 - 2)
                        rs_av(c0 - 1)
                rs_av(nKV - 2)
                rs_av(nKV - 1)
                pending = (ps_rs, ps_av, h, s4)

            flush_epilogue()  # head 3 of this s4 must land before Oproj(s4)

            for q4 in range(4):
                pss = quad_psum()
                for o in range(4):
                    for j in range(4):
                        nc.tensor.matmul(
                            pss[j],
                            wor[:, o, (q4 * 4 + j) * 128:(q4 * 4 + j + 1) * 128],
                            OT[:, o, s4 * 512:(s4 + 1) * 512],
                            start=(o == 0), stop=(o == 3), skip_group_check=True,
                        )
                for j in range(4):
                    ys = ystg.tile([P, 512], F16, tag="y")
                    nc.vector.tensor_copy(ys, pss[j])
                    nc.sync.dma_start(
                        out=yt_d[(q4 * 4 + j) * 128:(q4 * 4 + j + 1) * 128,
                                 s4 * 512:(s4 + 1) * 512],
                        in_=ys,
                    )

    nc.compile()
    return nc


_nc = None


def _get_nc():
    global _nc
    if _nc is None:
        _nc = build_nc()
    return _nc


def _pack_act(x):
    # [S, E] fp32 -> transposed slab layout [4(s4), P, nE, 512] fp16
    xt = np.asarray(x, np.float32).T.astype(np.float16)        # [E, S]
    return np.ascontiguousarray(
        xt.reshape(nE, P, 4, 512).transpose(2, 1, 0, 3)
    )


def _pack_w(w, ncols):
    # [E, ncols] fp32 -> [P, nE, ncols] fp16
    wh = np.asarray(w, np.float32).astype(np.float16)
    return np.ascontiguousarray(wh.reshape(nE, P, ncols).transpose(1, 0, 2))


def _make_in_maps(query, key, value, mask, Wq, Wk, Wv, Wo):
    qts = [_pack_act(query[b]) for b in range(B)]
    kts = [_pack_act(key[b]) for b in range(B)]
    vts = [_pack_act(value[b]) for b in range(B)]
    ms = [np.ascontiguousarray(mask[b], dtype=np.float32) for b in range(B)]
    wq_f, wk_f, wv_f, wo_f = (np.asarray(w, np.float32) for w in (Wq, Wk, Wv, Wo))
    in_maps = []
    for c in range(N_CORES):
        b, g = c // 4, c % 4
        wo_slice = wo_f[g * GQ:(g + 1) * GQ, :].astype(np.float16)  # [512, E]
        in_maps.append({
            "qt": qts[b],
            "kt": kts[b],
            "vt": vts[b],
            "m": ms[b],
            "wq": _pack_w(wq_f[:, g * GQ:(g + 1) * GQ], GQ),
            "wk": _pack_w(wk_f[:, g * GK:(g + 1) * GK], GK),
            "wv": _pack_w(wv_f[:, g * GK:(g + 1) * GK], GK),
            "wo": np.ascontiguousarray(
                wo_slice.reshape(4, P, E).transpose(1, 0, 2)
            ),
        })
    return in_maps


def run(query, key, value, mask, Wq, Wk, Wv, Wo, trace=False, trace_kwargs=None):
    from concourse.bass_utils import run_bass_kernel_spmd

    nc = _get_nc()
    in_maps = _make_in_maps(query, key, value, mask, Wq, Wk, Wv, Wo)
    res = run_bass_kernel_spmd(
        nc, in_maps, list(range(N_CORES)), trace=trace, **(trace_kwargs or {})
    )
    out = np.empty((B, S, E), np.float32)
    for b in range(B):
        acc = np.zeros((E, S), np.float32)
        for g in range(4):
            acc += res.results[b * 4 + g]["yt"].astype(np.float32)
        out[b] = acc.T
    return out, res


def kernel(query, key, value, mask, Wq, Wk, Wv, Wo):
    out, _ = run(query, key, value, mask, Wq, Wk, Wv, Wo, trace=False)
    return out


# revision 15
# speedup vs baseline: 1.7628x; 1.1091x over previous
"""GQA attention kernel for 8 Trainium2 NeuronCores.

Sharding: tensor-parallel over kv-head groups x data-parallel over batch.
Core c handles batch b = c//4 and kv-head group g = c%4 (query heads
4g..4g+3) for ALL 2048 query positions of its batch. Wq/Wk/Wv are split
column-wise by head group, Wo row-wise; each core emits a partial output
projection and the host sums the 4 partials per batch (the "all-reduce
after output projection" of classic TP, done on the host). This removes
the K/V-projection duplication that pure sequence-parallel pays.

All activations are kept feature-major on-chip; the host pre-transposes
AND pre-packs every streamed tensor into its exact SBUF slab layout, so
each DMA line is >=4KB-contiguous (full DMA bandwidth) and the kernel
contains NO PE transposes. All matmul operands are fp16 (1 cycle/row
streaming at N=512 -> ~216ns/matmul, the PE floor); PSUM accumulation
stays fp32. The host un-transposes + reduces the output. Rel err ~5e-3.

Pipeline (emission order interleaves DMA-hungry Kproj blocks between
Qproj/Vproj blocks so the DMA engines never starve the PE):

  Q0 K0 Q1 K1 V0 Q2 K2 V1 Q3 K3 V2 V3   (projections, PSUM-quad blocks)
  for s4 (query 512-block): 4 heads of attention, then Oproj(s4)

  attention per (head, s4), per PAIR of skv chunks:
    scoresT pair -> one [128,1024] PSUM pair        (2 matmuls)
    PT = exp(scoresT*scale)                         (one ACT op per pair)
    rowsum += maskcol.T @ PT ; OT[h] += Vn.T @ PT   (PSUM acc, 1-pair skew)
  OT *= 1/rowsum via exact fp32 rank-1 broadcast matmul, emitted lazily
  one iteration later so the PE never stalls on the epilogue.

The mask is applied by zeroing rows of Vn and using the mask itself as
the rowsum stationary vector (exp(-1e9)=0 equivalence), so the exp needs
no per-chunk bias and pairs of chunks share one ACT instruction.
"""

import os
import sys

sys.path.insert(0, "/opt/trn_rl_repo")
if os.environ.get("JAX_PLATFORMS") == "cpu":
    del os.environ["JAX_PLATFORMS"]
os.environ.setdefault("MYCRO_LOCAL_CACHE", "1")

from contextlib import ExitStack

import numpy as np

import concourse.bass as bass
import concourse.bacc as bacc
import concourse.mybir as mybir
import concourse.tile as tile

P = 128
E = 2048          # embed dim
S = 2048          # sequence length (queries and kv)
GQ = 512          # per-group query-projection width (4 heads * 128)
GK = 128          # per-group kv width (1 kv head)
nE = E // P       # 16
nKV = S // P      # 16
SC = 1.0 / float(128.0) ** 0.5
B = 2
N_CORES = 8

F32 = mybir.dt.float32
F16 = mybir.dt.float16
AF = mybir.ActivationFunctionType


def build_nc():
    nc = bacc.Bacc(target_bir_lowering=False)

    # activations prepacked on the host as [s4][p][e][512] slabs
    qt_d = nc.dram_tensor("qt", [4, P, nE, 512], F16, kind="ExternalInput")
    kt_d = nc.dram_tensor("kt", [4, P, nE, 512], F16, kind="ExternalInput")
    vt_d = nc.dram_tensor("vt", [4, P, nE, 512], F16, kind="ExternalInput")
    m_d = nc.dram_tensor("m", [S], F32, kind="ExternalInput")
    # weights prepacked as [p][e][cols]
    wq_d = nc.dram_tensor("wq", [P, nE, GQ], F16, kind="ExternalInput")
    wk_d = nc.dram_tensor("wk", [P, nE, GK], F16, kind="ExternalInput")
    wv_d = nc.dram_tensor("wv", [P, nE, GK], F16, kind="ExternalInput")
    wo_d = nc.dram_tensor("wo", [P, 4, E], F16, kind="ExternalInput")
    yt_d = nc.dram_tensor("yt", [E, S], F16, kind="ExternalOutput")

    with ExitStack() as ctx:
        tc = ctx.enter_context(tile.TileContext(nc))
        consts = ctx.enter_context(tc.tile_pool(name="consts", bufs=1))
        wqres = ctx.enter_context(tc.tile_pool(name="wqres", bufs=1))
        wkres = ctx.enter_context(tc.tile_pool(name="wkres", bufs=1))
        wvres = ctx.enter_context(tc.tile_pool(name="wvres", bufs=1))
        wores = ctx.enter_context(tc.tile_pool(name="wores", bufs=1))
        qslab = ctx.enter_context(tc.tile_pool(name="qslab", bufs=2))
        kvslab = ctx.enter_context(tc.tile_pool(name="kvslab", bufs=2))
        bigq = ctx.enter_context(tc.tile_pool(name="bigq", bufs=1))
        bigk = ctx.enter_context(tc.tile_pool(name="bigk", bufs=1))
        bigv = ctx.enter_context(tc.tile_pool(name="bigv", bufs=1))
        bigo = ctx.enter_context(tc.tile_pool(name="bigo", bufs=1))
        ptp = ctx.enter_context(tc.tile_pool(name="ptp", bufs=2))
        small = ctx.enter_context(tc.tile_pool(name="small", bufs=2))
        psmm = ctx.enter_context(tc.tile_pool(name="psmm", bufs=3, space="PSUM"))
        psra = ctx.enter_context(tc.tile_pool(name="psra", bufs=2, space="PSUM"))
        ystg = ctx.enter_context(tc.tile_pool(name="ystg", bufs=4))

        # ---- constants ----
        mask_sb = consts.tile([P, nKV], F32, tag="msk")
        nc.sync.dma_start(out=mask_sb, in_=m_d.rearrange("(a b) -> b a", b=P))
        mcol = consts.tile([P, nKV], F16, tag="mcol")
        nc.vector.tensor_copy(mcol, mask_sb)

        # ---- resident weights; wq interleaved with the first q slab so
        # ---- the PE starts streaming within ~3us of kernel start
        wqr = wqres.tile([P, nE, GQ], F16, tag="wq")
        QT = bigq.tile([P, 4, S], F16, tag="qt")
        qsl0 = qslab.tile([P, nE, 512], F16, tag="q", name="qsl0")
        for q in range(8):
            sl = slice(q * 2, (q + 1) * 2)
            nc.sync.dma_start(out=wqr[:, sl, :], in_=wq_d[:, sl, :])
            nc.sync.dma_start(out=qsl0[:, sl, :], in_=qt_d[0][:, sl, :])
        wkr = wkres.tile([P, nE, GK], F16, tag="wk")
        nc.sync.dma_start(out=wkr, in_=wk_d[:, :, :])
        wvr = wvres.tile([P, nE, GK], F16, tag="wv")
        nc.sync.dma_start(out=wvr, in_=wv_d[:, :, :])

        def load_slab(pool, src_d, s4, tag):
            sl = pool.tile([P, nE, 512], F16, tag=tag, name=f"slab_{tag}")
            for q in range(4):
                nc.sync.dma_start(
                    out=sl[:, q * 4:(q + 1) * 4, :],
                    in_=src_d[s4][:, q * 4:(q + 1) * 4, :],
                )
            return sl

        def quad_psum():
            prs = [psmm.tile([P, 2, 512], F32, tag="mm", name=f"pr{_i}")
                   for _i in range(2)]
            return [prs[_j // 2][:, _j % 2, :] for _j in range(4)]

        # ---- projection blocks ----
        KT = bigk.tile([P, S], F16, tag="kt")
        Vn = bigv.tile([P, nKV, GK], F16, tag="vn")

        def q_block(s4, qsl):
            pss = quad_psum()
            for e in range(nE):
                for hc in range(4):
                    nc.tensor.matmul(
                        pss[hc], wqr[:, e, hc * 128:(hc + 1) * 128], qsl[:, e, :],
                        start=(e == 0), stop=(e == nE - 1), skip_group_check=True,
                    )
            for hc in range(4):
                nc.vector.tensor_copy(QT[:, hc, s4 * 512:(s4 + 1) * 512], pss[hc])

        def k_block(cs):
            ksl = load_slab(kvslab, kt_d, cs, "kv")
            pr = psmm.tile([P, 2, 512], F32, tag="mm", name="prk")
            for e in range(nE):
                nc.tensor.matmul(
                    pr[:, 0, :], wkr[:, e, :], ksl[:, e, :],
                    start=(e == 0), stop=(e == nE - 1), skip_group_check=True,
                )
            nc.vector.tensor_copy(KT[:, cs * 512:(cs + 1) * 512], pr[:, 0, :])

        def v_block(mq):
            vsl = load_slab(kvslab, vt_d, mq, "kv")
            pss = quad_psum()
            for e in range(nE):
                for j in range(4):
                    nc.tensor.matmul(
                        pss[j][:, 0:GK], vsl[:, e, j * 128:(j + 1) * 128],
                        wvr[:, e, :],
                        start=(e == 0), stop=(e == nE - 1), skip_group_check=True,
                    )
            for j in range(4):
                c = mq * 4 + j
                # rows of V for masked skv positions are zeroed here
                nc.vector.tensor_scalar_mul(
                    Vn[:, c, :], pss[j][:, 0:GK], mask_sb[:, c:c + 1]
                )

        # interleave: K blocks are DMA-hungry (2MB per 3.5us of PE work),
        # so they sit between Q/V blocks and prefetch during them.
        q_block(0, qsl0)
        k_block(0)
        q_block(1, load_slab(qslab, qt_d, 1, "q"))
        k_block(1)
        v_block(0)
        q_block(2, load_slab(qslab, qt_d, 2, "q"))
        k_block(2)
        v_block(1)
        q_block(3, load_slab(qslab, qt_d, 3, "q"))
        k_block(3)
        v_block(2)
        v_block(3)

        # resident Wo: needed in Oproj, DMA hides under early attention
        wor = wores.tile([P, 4, E], F16, tag="wo")
        nc.sync.dma_start(out=wor, in_=wo_d[:, :, :])

        # ---- attention + output projection, query-block-major ----
        OT = bigo.tile([P, 4, S], F16, tag="ot")
        pending = None       # lazy epilogue: (ps_rs, ps_av, h, s4)
        pending_tail = None  # previous iteration's last rs/av pair

        def flush_tail():
            nonlocal pending_tail
            if pending_tail is not None:
                pending_tail()
                pending_tail = None

        def flush_epilogue():
            nonlocal pending
            if pending is None:
                return
            ps_rs, ps_av, h, s4 = pending
            pending = None
            # reciprocal of the [1,512] denominator, broadcast to all 128
            # partitions on the (otherwise idle) GpSimd engine -- no PE work
            recip_sm = small.tile([1, 512], F32, tag="recip_sm")
            nc.vector.reciprocal_approx_fast(out=recip_sm, in_=ps_rs)
            recip_bc = small.tile([P, 512], F32, tag="recip_bc")
            nc.gpsimd.partition_broadcast(recip_bc, recip_sm, channels=P)
            nc.vector.tensor_mul(
                OT[:, h, s4 * 512:(s4 + 1) * 512], ps_av, recip_bc
            )

        for s4 in range(4):
            for h in range(4):
                qs = QT[:, h, s4 * 512:(s4 + 1) * 512]
                ps_rs = psra.tile([1, 512], F32, tag="ra")
                ps_av = psra.tile([P, 512], F32, tag="ra")
                PTh = [None, None]

                def rs_av(c, PTh=PTh, ps_rs=ps_rs, ps_av=ps_av):
                    pt_c = PTh[c // 8][:, c % 8, :]
                    nc.tensor.matmul(
                        ps_rs, mcol[:, c:c + 1], pt_c,
                        start=(c == 0), stop=(c == nKV - 1),
                        skip_group_check=True,
                    )
                    nc.tensor.matmul(
                        ps_av, Vn[:, c, :], pt_c,
                        start=(c == 0), stop=(c == nKV - 1),
                        skip_group_check=True,
                    )

                for p in range(nKV // 2):  # pairs of skv chunks
                    c0 = 2 * p
                    if c0 % 8 == 0:
                        PTh[c0 // 8] = ptp.tile([P, 8, 512], F16, tag="pt",
                                                name="PTh")
                    ps_s = psmm.tile([P, 2, 512], F32, tag="mm", name="ps_s")
                    for i in range(2):
                        nc.tensor.matmul(
                            ps_s[:, i, :],
                            KT[:, (c0 + i) * 128:(c0 + i + 1) * 128],
                            qs, start=True, stop=True,
                        )
                    if p == 0:
                        # previous iteration's last rs/av pair lands under
                        # this iteration's first exp latency
                        flush_tail()
                    if p == 1:
                        flush_epilogue()
                    nc.scalar.activation(
                        PTh[c0 // 8][:, c0 % 8:c0 % 8 + 2, :], ps_s, AF.Exp,
                        scale=SC,
                    )
                    if p >= 1:
                        rs_av(c0 - 2)
                        rs_av(c0 - 1)
                pending_tail = lambda rs_av=rs_av: (rs_av(nKV - 2),
                                                    rs_av(nKV - 1))
                pending = (ps_rs, ps_av, h, s4)

            # head 3 of this s4 must fully land before Oproj(s4)
            flush_tail()
            flush_epilogue()

            for q4 in range(4):
                pss = quad_psum()
                for o in range(4):
                    for j in range(4):
                        nc.tensor.matmul(
                            pss[j],
                            wor[:, o, (q4 * 4 + j) * 128:(q4 * 4 + j + 1) * 128],
                            OT[:, o, s4 * 512:(s4 + 1) * 512],
                            start=(o == 0), stop=(o == 3), skip_group_check=True,
                        )
                for j in range(4):
                    ys = ystg.tile([P, 512], F16, tag="y")
                    nc.vector.tensor_copy(ys, pss[j])
                    nc.sync.dma_start(
                        out=yt_d[(q4 * 4 + j) * 128:(q4 * 4 + j + 1) * 128,
                                 s4 * 512:(s4 + 1) * 512],
                        in_=ys,
                    )

    nc.compile()
    return nc


_nc = None


def _get_nc():
    global _nc
    if _nc is None:
        _nc = build_nc()
    return _nc


def _pack_act(x):
    # [S, E] fp32 -> transposed slab layout [4(s4), P, nE, 512] fp16
    xt = np.asarray(x, np.float32).T.astype(np.float16)        # [E, S]
    return np.ascontiguousarray(
        xt.reshape(nE, P, 4, 512).transpose(2, 1, 0, 3)
    )


def _pack_w(w, ncols):
    # [E, ncols] fp32 -> [P, nE, ncols] fp16
    wh = np.asarray(w, np.float32).astype(np.float16)
    return np.ascontiguousarray(wh.reshape(nE, P, ncols).transpose(1, 0, 2))


def _make_in_maps(query, key, value, mask, Wq, Wk, Wv, Wo):
    qts = [_pack_act(query[b]) for b in range(B)]
    kts = [_pack_act(key[b]) for b in range(B)]
    vts = [_pack_act(value[b]) for b in range(B)]
    ms = [np.ascontiguousarray(mask[b], dtype=np.float32) for b in range(B)]
    wq_f, wk_f, wv_f, wo_f = (np.asarray(w, np.float32) for w in (Wq, Wk, Wv, Wo))
    in_maps = []
    for c in range(N_CORES):
        b, g = c // 4, c % 4
        wo_slice = wo_f[g * GQ:(g + 1) * GQ, :].astype(np.float16)  # [512, E]
        in_maps.append({
            "qt": qts[b],
            "kt": kts[b],
            "vt": vts[b],
            "m": ms[b],
            "wq": _pack_w(wq_f[:, g * GQ:(g + 1) * GQ], GQ),
            "wk": _pack_w(wk_f[:, g * GK:(g + 1) * GK], GK),
            "wv": _pack_w(wv_f[:, g * GK:(g + 1) * GK], GK),
            "wo": np.ascontiguousarray(
                wo_slice.reshape(4, P, E).transpose(1, 0, 2)
            ),
        })
    return in_maps


def run(query, key, value, mask, Wq, Wk, Wv, Wo, trace=False, trace_kwargs=None):
    from concourse.bass_utils import run_bass_kernel_spmd

    nc = _get_nc()
    in_maps = _make_in_maps(query, key, value, mask, Wq, Wk, Wv, Wo)
    res = run_bass_kernel_spmd(
        nc, in_maps, list(range(N_CORES)), trace=trace, **(trace_kwargs or {})
    )
    out = np.empty((B, S, E), np.float32)
    for b in range(B):
        acc = np.zeros((E, S), np.float32)
        for g in range(4):
            acc += res.results[b * 4 + g]["yt"].astype(np.float32)
        out[b] = acc.T
    return out, res


def kernel(query, key, value, mask, Wq, Wk, Wv, Wo):
    out, _ = run(query, key, value, mask, Wq, Wk, Wv, Wo, trace=False)
    return out


# revision 17
# speedup vs baseline: 1.7655x; 1.0015x over previous
"""GQA attention kernel for 8 Trainium2 NeuronCores.

Sharding: tensor-parallel over kv-head groups x data-parallel over batch.
Core c handles batch b = c//4 and kv-head group g = c%4 (query heads
4g..4g+3) for ALL 2048 query positions of its batch. Wq/Wk/Wv are split
column-wise by head group, Wo row-wise; each core emits a partial output
projection and the host sums the 4 partials per batch (the "all-reduce
after output projection" of classic TP, done on the host). This removes
the K/V-projection duplication that pure sequence-parallel pays.

All activations are kept feature-major on-chip; the host pre-transposes
AND pre-packs every streamed tensor into its exact SBUF slab layout, so
each DMA line is >=4KB-contiguous (full DMA bandwidth) and the kernel
contains NO PE transposes. All matmul operands are fp16 (1 cycle/row
streaming at N=512 -> ~216ns/matmul, the PE floor); PSUM accumulation
stays fp32. The host un-transposes + reduces the output. Rel err ~5e-3.

Pipeline (emission order interleaves DMA-hungry Kproj blocks between
Qproj/Vproj blocks so the DMA engines never starve the PE):

  Q0 K0 Q1 K1 V0 Q2 K2 V1 Q3 K3 V2 V3   (projections, PSUM-quad blocks)
  for s4 (query 512-block): 4 heads of attention, then Oproj(s4)

  attention per (head, s4), per PAIR of skv chunks:
    scoresT pair -> one [128,1024] PSUM pair        (2 matmuls)
    PT = exp(scoresT*scale)                         (one ACT op per pair)
    rowsum += maskcol.T @ PT ; OT[h] += Vn.T @ PT   (PSUM acc, 1-pair skew)
  OT *= 1/rowsum via exact fp32 rank-1 broadcast matmul, emitted lazily
  one iteration later so the PE never stalls on the epilogue.

The mask is applied by zeroing rows of Vn and using the mask itself as
the rowsum stationary vector (exp(-1e9)=0 equivalence), so the exp needs
no per-chunk bias and pairs of chunks share one ACT instruction.
"""

import os
import sys

sys.path.insert(0, "/opt/trn_rl_repo")
if os.environ.get("JAX_PLATFORMS") == "cpu":
    del os.environ["JAX_PLATFORMS"]
os.environ.setdefault("MYCRO_LOCAL_CACHE", "1")

from contextlib import ExitStack

import numpy as np

import concourse.bass as bass
import concourse.bacc as bacc
import concourse.mybir as mybir
import concourse.tile as tile

P = 128
E = 2048          # embed dim
S = 2048          # sequence length (queries and kv)
GQ = 512          # per-group query-projection width (4 heads * 128)
GK = 128          # per-group kv width (1 kv head)
nE = E // P       # 16
nKV = S // P      # 16
SC = 1.0 / float(128.0) ** 0.5
B = 2
N_CORES = 8

F32 = mybir.dt.float32
F16 = mybir.dt.float16
AF = mybir.ActivationFunctionType


def build_nc():
    nc = bacc.Bacc(target_bir_lowering=False)

    # activations prepacked on the host as [s4][p][e][512] slabs
    qt_d = nc.dram_tensor("qt", [4, P, nE, 512], F16, kind="ExternalInput")
    kt_d = nc.dram_tensor("kt", [4, P, nE, 512], F16, kind="ExternalInput")
    vt_d = nc.dram_tensor("vt", [4, P, nE, 512], F16, kind="ExternalInput")
    m_d = nc.dram_tensor("m", [S], F32, kind="ExternalInput")
    # weights prepacked as [p][e][cols]
    wq_d = nc.dram_tensor("wq", [P, nE, GQ], F16, kind="ExternalInput")
    wk_d = nc.dram_tensor("wk", [P, nE, GK], F16, kind="ExternalInput")
    wv_d = nc.dram_tensor("wv", [P, nE, GK], F16, kind="ExternalInput")
    wo_d = nc.dram_tensor("wo", [P, 4, E], F16, kind="ExternalInput")
    yt_d = nc.dram_tensor("yt", [E, S], F16, kind="ExternalOutput")

    with ExitStack() as ctx:
        tc = ctx.enter_context(tile.TileContext(nc))
        consts = ctx.enter_context(tc.tile_pool(name="consts", bufs=1))
        wqres = ctx.enter_context(tc.tile_pool(name="wqres", bufs=1))
        wkres = ctx.enter_context(tc.tile_pool(name="wkres", bufs=1))
        wvres = ctx.enter_context(tc.tile_pool(name="wvres", bufs=1))
        wores = ctx.enter_context(tc.tile_pool(name="wores", bufs=1))
        qslab = ctx.enter_context(tc.tile_pool(name="qslab", bufs=2))
        kvslab = ctx.enter_context(tc.tile_pool(name="kvslab", bufs=3))
        bigq = ctx.enter_context(tc.tile_pool(name="bigq", bufs=1))
        bigk = ctx.enter_context(tc.tile_pool(name="bigk", bufs=1))
        bigv = ctx.enter_context(tc.tile_pool(name="bigv", bufs=1))
        bigo = ctx.enter_context(tc.tile_pool(name="bigo", bufs=1))
        ptp = ctx.enter_context(tc.tile_pool(name="ptp", bufs=2))
        small = ctx.enter_context(tc.tile_pool(name="small", bufs=2))
        psmm = ctx.enter_context(tc.tile_pool(name="psmm", bufs=3, space="PSUM"))
        psra = ctx.enter_context(tc.tile_pool(name="psra", bufs=2, space="PSUM"))
        ystg = ctx.enter_context(tc.tile_pool(name="ystg", bufs=4))

        # ---- constants ----
        mask_sb = consts.tile([P, nKV], F32, tag="msk")
        nc.sync.dma_start(out=mask_sb, in_=m_d.rearrange("(a b) -> b a", b=P))
        mcol = consts.tile([P, nKV], F16, tag="mcol")
        nc.vector.tensor_copy(mcol, mask_sb)

        # ---- resident weights; wq interleaved with the first q slab so
        # ---- the PE starts streaming within ~3us of kernel start
        wqr = wqres.tile([P, nE, GQ], F16, tag="wq")
        QT = bigq.tile([P, 4, S], F16, tag="qt")
        qsl0 = qslab.tile([P, nE, 512], F16, tag="q", name="qsl0")
        # 1-e granularity for the first tiles so the PE starts ~1us in,
        # then coarser chunks for efficiency
        for sl in ([slice(e, e + 1) for e in range(4)]
                   + [slice(4 + 4 * q, 8 + 4 * q) for q in range(3)]):
            nc.sync.dma_start(out=wqr[:, sl, :], in_=wq_d[:, sl, :])
            nc.sync.dma_start(out=qsl0[:, sl, :], in_=qt_d[0][:, sl, :])
        wkr = wkres.tile([P, nE, GK], F16, tag="wk")
        nc.sync.dma_start(out=wkr, in_=wk_d[:, :, :])
        wvr = wvres.tile([P, nE, GK], F16, tag="wv")
        nc.sync.dma_start(out=wvr, in_=wv_d[:, :, :])

        def load_slab(pool, src_d, s4, tag):
            sl = pool.tile([P, nE, 512], F16, tag=tag, name=f"slab_{tag}")
            for q in range(4):
                nc.sync.dma_start(
                    out=sl[:, q * 4:(q + 1) * 4, :],
                    in_=src_d[s4][:, q * 4:(q + 1) * 4, :],
                )
            return sl

        def quad_psum():
            prs = [psmm.tile([P, 2, 512], F32, tag="mm", name=f"pr{_i}")
                   for _i in range(2)]
            return [prs[_j // 2][:, _j % 2, :] for _j in range(4)]

        # ---- projection blocks ----
        KT = bigk.tile([P, S], F16, tag="kt")
        Vn = bigv.tile([P, nKV, GK], F16, tag="vn")

        def q_block(s4, qsl):
            pss = quad_psum()
            for e in range(nE):
                for hc in range(4):
                    nc.tensor.matmul(
                        pss[hc], wqr[:, e, hc * 128:(hc + 1) * 128], qsl[:, e, :],
                        start=(e == 0), stop=(e == nE - 1), skip_group_check=True,
                    )
            for hc in range(4):
                nc.vector.tensor_copy(QT[:, hc, s4 * 512:(s4 + 1) * 512], pss[hc])

        def k_block(cs):
            ksl = load_slab(kvslab, kt_d, cs, "kv")
            pr = psmm.tile([P, 2, 512], F32, tag="mm", name="prk")
            for e in range(nE):
                nc.tensor.matmul(
                    pr[:, 0, :], wkr[:, e, :], ksl[:, e, :],
                    start=(e == 0), stop=(e == nE - 1), skip_group_check=True,
                )
            nc.vector.tensor_copy(KT[:, cs * 512:(cs + 1) * 512], pr[:, 0, :])

        def v_block(mq):
            vsl = load_slab(kvslab, vt_d, mq, "kv")
            pss = quad_psum()
            for e in range(nE):
                for j in range(4):
                    nc.tensor.matmul(
                        pss[j][:, 0:GK], vsl[:, e, j * 128:(j + 1) * 128],
                        wvr[:, e, :],
                        start=(e == 0), stop=(e == nE - 1), skip_group_check=True,
                    )
            for j in range(4):
                c = mq * 4 + j
                # rows of V for masked skv positions are zeroed here
                nc.vector.tensor_scalar_mul(
                    Vn[:, c, :], pss[j][:, 0:GK], mask_sb[:, c:c + 1]
                )

        # interleave: K blocks are DMA-hungry (2MB per 3.5us of PE work),
        # so they sit between Q/V blocks and prefetch during them.
        q_block(0, qsl0)
        k_block(0)
        q_block(1, load_slab(qslab, qt_d, 1, "q"))
        k_block(1)
        v_block(0)
        q_block(2, load_slab(qslab, qt_d, 2, "q"))
        k_block(2)
        v_block(1)
        q_block(3, load_slab(qslab, qt_d, 3, "q"))
        k_block(3)
        v_block(2)
        v_block(3)

        # resident Wo: needed in Oproj, DMA hides under early attention
        wor = wores.tile([P, 4, E], F16, tag="wo")
        nc.sync.dma_start(out=wor, in_=wo_d[:, :, :])

        # ---- attention + output projection, query-block-major ----
        OT = bigo.tile([P, 4, S], F16, tag="ot")
        pending = None       # lazy epilogue: (ps_rs, ps_av, h, s4)
        pending_tail = None  # previous iteration's last rs/av pair

        def flush_tail():
            nonlocal pending_tail
            if pending_tail is not None:
                pending_tail()
                pending_tail = None

        def flush_epilogue():
            nonlocal pending
            if pending is None:
                return
            ps_rs, ps_av, h, s4 = pending
            pending = None
            # reciprocal of the [1,512] denominator, broadcast to all 128
            # partitions on the (otherwise idle) GpSimd engine -- no PE work
            recip_sm = small.tile([1, 512], F32, tag="recip_sm")
            nc.vector.reciprocal_approx_fast(out=recip_sm, in_=ps_rs)
            recip_bc = small.tile([P, 512], F32, tag="recip_bc")
            nc.gpsimd.partition_broadcast(recip_bc, recip_sm, channels=P)
            nc.vector.tensor_mul(
                OT[:, h, s4 * 512:(s4 + 1) * 512], ps_av, recip_bc
            )

        for s4 in range(4):
            for h in range(4):
                qs = QT[:, h, s4 * 512:(s4 + 1) * 512]
                ps_rs = psra.tile([1, 512], F32, tag="ra")
                ps_av = psra.tile([P, 512], F32, tag="ra")
                PTh = [None, None]

                def rs_av(c, PTh=PTh, ps_rs=ps_rs, ps_av=ps_av):
                    pt_c = PTh[c // 8][:, c % 8, :]
                    nc.tensor.matmul(
                        ps_rs, mcol[:, c:c + 1], pt_c,
                        start=(c == 0), stop=(c == nKV - 1),
                        skip_group_check=True,
                    )
                    nc.tensor.matmul(
                        ps_av, Vn[:, c, :], pt_c,
                        start=(c == 0), stop=(c == nKV - 1),
                        skip_group_check=True,
                    )

                for p in range(nKV // 2):  # pairs of skv chunks
                    c0 = 2 * p
                    if c0 % 8 == 0:
                        PTh[c0 // 8] = ptp.tile([P, 8, 512], F16, tag="pt",
                                                name="PTh")
                    ps_s = psmm.tile([P, 2, 512], F32, tag="mm", name="ps_s")
                    for i in range(2):
                        nc.tensor.matmul(
                            ps_s[:, i, :],
                            KT[:, (c0 + i) * 128:(c0 + i + 1) * 128],
                            qs, start=True, stop=True,
                        )
                    if p == 0:
                        # previous iteration's last rs/av pair lands under
                        # this iteration's first exp latency
                        flush_tail()
                    if p == 1:
                        flush_epilogue()
                    nc.scalar.activation(
                        PTh[c0 // 8][:, c0 % 8:c0 % 8 + 2, :], ps_s, AF.Exp,
                        scale=SC,
                    )
                    if p >= 1:
                        rs_av(c0 - 2)
                        rs_av(c0 - 1)
                pending_tail = lambda rs_av=rs_av: (rs_av(nKV - 2),
                                                    rs_av(nKV - 1))
                pending = (ps_rs, ps_av, h, s4)

            # head 3 of this s4 must fully land before Oproj(s4)
            flush_tail()
            flush_epilogue()

            for q4 in range(4):
                pss = quad_psum()
                for o in range(4):
                    for j in range(4):
                        nc.tensor.matmul(
                            pss[j],
                            wor[:, o, (q4 * 4 + j) * 128:(q4 * 4 + j + 1) * 128],
                            OT[:, o, s4 * 512:(s4 + 1) * 512],
                            start=(o == 0), stop=(o == 3), skip_group_check=True,
                        )
                for j in range(4):
                    ys = ystg.tile([P, 512], F16, tag="y")
                    nc.vector.tensor_copy(ys, pss[j])
                    nc.sync.dma_start(
                        out=yt_d[(q4 * 4 + j) * 128:(q4 * 4 + j + 1) * 128,
                                 s4 * 512:(s4 + 1) * 512],
                        in_=ys,
                    )

    nc.compile()
    return nc


_nc = None


def _get_nc():
    global _nc
    if _nc is None:
        _nc = build_nc()
    return _nc


def _pack_act(x):
    # [S, E] fp32 -> transposed slab layout [4(s4), P, nE, 512] fp16
    xt = np.asarray(x, np.float32).T.astype(np.float16)        # [E, S]
    return np.ascontiguousarray(
        xt.reshape(nE, P, 4, 512).transpose(2, 1, 0, 3)
    )


def _pack_w(w, ncols):
    # [E, ncols] fp32 -> [P, nE, ncols] fp16
    wh = np.asarray(w, np.float32).astype(np.float16)
    return np.ascontiguousarray(wh.reshape(nE, P, ncols).transpose(1, 0, 2))


def _make_in_maps(query, key, value, mask, Wq, Wk, Wv, Wo):
    qts = [_pack_act(query[b]) for b in range(B)]
    kts = [_pack_act(key[b]) for b in range(B)]
    vts = [_pack_act(value[b]) for b in range(B)]
    ms = [np.ascontiguousarray(mask[b], dtype=np.float32) for b in range(B)]
    wq_f, wk_f, wv_f, wo_f = (np.asarray(w, np.float32) for w in (Wq, Wk, Wv, Wo))
    in_maps = []
    for c in range(N_CORES):
        b, g = c // 4, c % 4
        wo_slice = wo_f[g * GQ:(g + 1) * GQ, :].astype(np.float16)  # [512, E]
        in_maps.append({
            "qt": qts[b],
            "kt": kts[b],
            "vt": vts[b],
            "m": ms[b],
            "wq": _pack_w(wq_f[:, g * GQ:(g + 1) * GQ], GQ),
            "wk": _pack_w(wk_f[:, g * GK:(g + 1) * GK], GK),
            "wv": _pack_w(wv_f[:, g * GK:(g + 1) * GK], GK),
            "wo": np.ascontiguousarray(
                wo_slice.reshape(4, P, E).transpose(1, 0, 2)
            ),
        })
    return in_maps


def run(query, key, value, mask, Wq, Wk, Wv, Wo, trace=False, trace_kwargs=None):
    from concourse.bass_utils import run_bass_kernel_spmd

    nc = _get_nc()
    in_maps = _make_in_maps(query, key, value, mask, Wq, Wk, Wv, Wo)
    res = run_bass_kernel_spmd(
        nc, in_maps, list(range(N_CORES)), trace=trace, **(trace_kwargs or {})
    )
    out = np.empty((B, S, E), np.float32)
    for b in range(B):
        acc = np.zeros((E, S), np.float32)
        for g in range(4):
            acc += res.results[b * 4 + g]["yt"].astype(np.float32)
        out[b] = acc.T
    return out, res


def kernel(query, key, value, mask, Wq, Wk, Wv, Wo):
    out, _ = run(query, key, value, mask, Wq, Wk, Wv, Wo, trace=False)
    return out


# revision 19
# speedup vs baseline: 1.7709x; 1.0031x over previous
"""GQA attention kernel for 8 Trainium2 NeuronCores.

Sharding: tensor-parallel over kv-head groups x data-parallel over batch.
Core c handles batch b = c//4 and kv-head group g = c%4 (query heads
4g..4g+3) for ALL 2048 query positions of its batch. Wq/Wk/Wv are split
column-wise by head group, Wo row-wise; each core emits a partial output
projection and the host sums the 4 partials per batch (the "all-reduce
after output projection" of classic TP, done on the host). This removes
the K/V-projection duplication that pure sequence-parallel pays.

All activations are kept feature-major on-chip; the host pre-transposes
AND pre-packs every streamed tensor into its exact SBUF slab layout, so
each DMA line is >=4KB-contiguous (full DMA bandwidth) and the kernel
contains NO PE transposes. All matmul operands are fp16 (1 cycle/row
streaming at N=512 -> ~216ns/matmul, the PE floor); PSUM accumulation
stays fp32. The host un-transposes + reduces the output. Rel err ~5e-3.

Pipeline (emission order interleaves DMA-hungry Kproj blocks between
Qproj/Vproj blocks so the DMA engines never starve the PE):

  Q0 K0 Q1 K1 V0 Q2 K2 V1 Q3 K3 V2 V3   (projections, PSUM-quad blocks)
  for s4 (query 512-block): 4 heads of attention, then Oproj(s4)

  attention per (head, s4), per PAIR of skv chunks:
    scoresT pair -> one [128,1024] PSUM pair        (2 matmuls)
    PT = exp(scoresT*scale)                         (one ACT op per pair)
    rowsum += maskcol.T @ PT ; OT[h] += Vn.T @ PT   (PSUM acc, 1-pair skew)
  OT *= 1/rowsum via exact fp32 rank-1 broadcast matmul, emitted lazily
  one iteration later so the PE never stalls on the epilogue.

The mask is applied by zeroing rows of Vn and using the mask itself as
the rowsum stationary vector (exp(-1e9)=0 equivalence), so the exp needs
no per-chunk bias and pairs of chunks share one ACT instruction.
"""

import os
import sys

sys.path.insert(0, "/opt/trn_rl_repo")
if os.environ.get("JAX_PLATFORMS") == "cpu":
    del os.environ["JAX_PLATFORMS"]
os.environ.setdefault("MYCRO_LOCAL_CACHE", "1")

from contextlib import ExitStack

import numpy as np

import concourse.bass as bass
import concourse.bacc as bacc
import concourse.mybir as mybir
import concourse.tile as tile

P = 128
E = 2048          # embed dim
S = 2048          # sequence length (queries and kv)
GQ = 512          # per-group query-projection width (4 heads * 128)
GK = 128          # per-group kv width (1 kv head)
nE = E // P       # 16
nKV = S // P      # 16
SC = 1.0 / float(128.0) ** 0.5
B = 2
N_CORES = 8

F32 = mybir.dt.float32
F16 = mybir.dt.float16
AF = mybir.ActivationFunctionType


def build_nc():
    nc = bacc.Bacc(target_bir_lowering=False)

    # activations prepacked on the host as [s4][p][e][512] slabs
    qt_d = nc.dram_tensor("qt", [4, P, nE, 512], F16, kind="ExternalInput")
    kt_d = nc.dram_tensor("kt", [4, P, nE, 512], F16, kind="ExternalInput")
    vt_d = nc.dram_tensor("vt", [4, P, nE, 512], F16, kind="ExternalInput")
    m_d = nc.dram_tensor("m", [S], F32, kind="ExternalInput")
    # weights prepacked as [p][e][cols]
    wq_d = nc.dram_tensor("wq", [P, nE, GQ], F16, kind="ExternalInput")
    wk_d = nc.dram_tensor("wk", [P, nE, GK], F16, kind="ExternalInput")
    wv_d = nc.dram_tensor("wv", [P, nE, GK], F16, kind="ExternalInput")
    wo_d = nc.dram_tensor("wo", [P, 4, E], F16, kind="ExternalInput")
    yt_d = nc.dram_tensor("yt", [E, S], F16, kind="ExternalOutput")

    with ExitStack() as ctx:
        tc = ctx.enter_context(tile.TileContext(nc))
        consts = ctx.enter_context(tc.tile_pool(name="consts", bufs=1))
        wqres = ctx.enter_context(tc.tile_pool(name="wqres", bufs=1))
        wkres = ctx.enter_context(tc.tile_pool(name="wkres", bufs=1))
        wvres = ctx.enter_context(tc.tile_pool(name="wvres", bufs=1))
        wores = ctx.enter_context(tc.tile_pool(name="wores", bufs=1))
        qslab = ctx.enter_context(tc.tile_pool(name="qslab", bufs=2))
        kvslab = ctx.enter_context(tc.tile_pool(name="kvslab", bufs=3))
        bigq = ctx.enter_context(tc.tile_pool(name="bigq", bufs=1))
        bigk = ctx.enter_context(tc.tile_pool(name="bigk", bufs=1))
        bigv = ctx.enter_context(tc.tile_pool(name="bigv", bufs=1))
        bigo = ctx.enter_context(tc.tile_pool(name="bigo", bufs=1))
        ptp = ctx.enter_context(tc.tile_pool(name="ptp", bufs=2))
        small = ctx.enter_context(tc.tile_pool(name="small", bufs=2))
        psmm = ctx.enter_context(tc.tile_pool(name="psmm", bufs=3, space="PSUM"))
        psra = ctx.enter_context(tc.tile_pool(name="psra", bufs=2, space="PSUM"))
        ystg = ctx.enter_context(tc.tile_pool(name="ystg", bufs=4))

        # ---- constants ----
        mask_sb = consts.tile([P, nKV], F32, tag="msk")
        nc.sync.dma_start(out=mask_sb, in_=m_d.rearrange("(a b) -> b a", b=P))
        mcol = consts.tile([P, nKV], F16, tag="mcol")
        nc.vector.tensor_copy(mcol, mask_sb)

        # ---- resident weights; wq interleaved with the first q slab so
        # ---- the PE starts streaming within ~3us of kernel start
        wqr = wqres.tile([P, nE, GQ], F16, tag="wq")
        QT = bigq.tile([P, 4, S], F16, tag="qt")
        qsl0 = qslab.tile([P, nE, 512], F16, tag="q", name="qsl0")
        # 1-e granularity for the first tiles so the PE starts ~1us in,
        # then coarser chunks for efficiency
        for sl in ([slice(e, e + 1) for e in range(4)]
                   + [slice(4 + 4 * q, 8 + 4 * q) for q in range(3)]):
            nc.sync.dma_start(out=wqr[:, sl, :], in_=wq_d[:, sl, :])
            nc.sync.dma_start(out=qsl0[:, sl, :], in_=qt_d[0][:, sl, :])
        wkr = wkres.tile([P, nE, GK], F16, tag="wk")
        nc.sync.dma_start(out=wkr, in_=wk_d[:, :, :])
        wvr = wvres.tile([P, nE, GK], F16, tag="wv")
        nc.sync.dma_start(out=wvr, in_=wv_d[:, :, :])

        def load_slab(pool, src_d, s4, tag):
            sl = pool.tile([P, nE, 512], F16, tag=tag, name=f"slab_{tag}")
            for q in range(4):
                nc.sync.dma_start(
                    out=sl[:, q * 4:(q + 1) * 4, :],
                    in_=src_d[s4][:, q * 4:(q + 1) * 4, :],
                )
            return sl

        def quad_psum():
            prs = [psmm.tile([P, 2, 512], F32, tag="mm", name=f"pr{_i}")
                   for _i in range(2)]
            return [prs[_j // 2][:, _j % 2, :] for _j in range(4)]

        # ---- projection blocks ----
        KT = bigk.tile([P, S], F16, tag="kt")
        Vn = bigv.tile([P, nKV, GK], F16, tag="vn")

        def q_block(s4, qsl):
            pss = quad_psum()
            for e in range(nE):
                for hc in range(4):
                    nc.tensor.matmul(
                        pss[hc], wqr[:, e, hc * 128:(hc + 1) * 128], qsl[:, e, :],
                        start=(e == 0), stop=(e == nE - 1), skip_group_check=True,
                    )
            for hc in range(4):
                nc.vector.tensor_copy(QT[:, hc, s4 * 512:(s4 + 1) * 512], pss[hc])

        def k_block(cs):
            ksl = load_slab(kvslab, kt_d, cs, "kv")
            pr = psmm.tile([P, 2, 512], F32, tag="mm", name="prk")
            for e in range(nE):
                nc.tensor.matmul(
                    pr[:, 0, :], wkr[:, e, :], ksl[:, e, :],
                    start=(e == 0), stop=(e == nE - 1), skip_group_check=True,
                )
            nc.vector.tensor_copy(KT[:, cs * 512:(cs + 1) * 512], pr[:, 0, :])

        def v_block(mq):
            vsl = load_slab(kvslab, vt_d, mq, "kv")
            pss = quad_psum()
            for e in range(nE):
                for j in range(4):
                    nc.tensor.matmul(
                        pss[j][:, 0:GK], vsl[:, e, j * 128:(j + 1) * 128],
                        wvr[:, e, :],
                        start=(e == 0), stop=(e == nE - 1), skip_group_check=True,
                    )
            for j in range(4):
                c = mq * 4 + j
                # rows of V for masked skv positions are zeroed here
                nc.vector.tensor_scalar_mul(
                    Vn[:, c, :], pss[j][:, 0:GK], mask_sb[:, c:c + 1]
                )

        # interleave: K blocks are DMA-hungry (2MB per 3.5us of PE work),
        # so they sit between Q/V blocks and prefetch during them.
        q_block(0, qsl0)
        k_block(0)
        q_block(1, load_slab(qslab, qt_d, 1, "q"))
        k_block(1)
        v_block(0)
        q_block(2, load_slab(qslab, qt_d, 2, "q"))
        k_block(2)
        v_block(1)
        q_block(3, load_slab(qslab, qt_d, 3, "q"))
        k_block(3)
        v_block(2)
        v_block(3)

        # resident Wo: needed in Oproj, DMA hides under early attention
        wor = wores.tile([P, 4, E], F16, tag="wo")
        nc.sync.dma_start(out=wor, in_=wo_d[:, :, :])

        # ---- attention + output projection, query-block-major ----
        OT = bigo.tile([P, 4, S], F16, tag="ot")
        pending = None        # lazy epilogue: (ps_rs, ps_av, h, s4)
        pending_tails = []    # previous iteration's last two rs/av pairs

        def flush_tail():
            if pending_tails:
                pending_tails.pop(0)()

        def flush_epilogue():
            nonlocal pending
            if pending is None:
                return
            ps_rs, ps_av, h, s4 = pending
            pending = None
            # reciprocal of the [1,512] denominator, broadcast to all 128
            # partitions on the (otherwise idle) GpSimd engine -- no PE work
            recip_sm = small.tile([1, 512], F32, tag="recip_sm")
            nc.vector.reciprocal_approx_fast(out=recip_sm, in_=ps_rs)
            recip_bc = small.tile([P, 512], F32, tag="recip_bc")
            nc.gpsimd.partition_broadcast(recip_bc, recip_sm, channels=P)
            nc.vector.tensor_mul(
                OT[:, h, s4 * 512:(s4 + 1) * 512], ps_av, recip_bc
            )

        for s4 in range(4):
            for h in range(4):
                qs = QT[:, h, s4 * 512:(s4 + 1) * 512]
                ps_rs = psra.tile([1, 512], F32, tag="ra")
                ps_av = psra.tile([P, 512], F32, tag="ra")
                PTh = [None, None]

                def rs_av(c, PTh=PTh, ps_rs=ps_rs, ps_av=ps_av):
                    pt_c = PTh[c // 8][:, c % 8, :]
                    nc.tensor.matmul(
                        ps_rs, mcol[:, c:c + 1], pt_c,
                        start=(c == 0), stop=(c == nKV - 1),
                        skip_group_check=True,
                    )
                    nc.tensor.matmul(
                        ps_av, Vn[:, c, :], pt_c,
                        start=(c == 0), stop=(c == nKV - 1),
                        skip_group_check=True,
                    )

                for p in range(nKV // 2):  # pairs of skv chunks
                    c0 = 2 * p
                    if c0 % 8 == 0:
                        PTh[c0 // 8] = ptp.tile([P, 8, 512], F16, tag="pt",
                                                name="PTh")
                    ps_s = psmm.tile([P, 2, 512], F32, tag="mm", name="ps_s")
                    for i in range(2):
                        nc.tensor.matmul(
                            ps_s[:, i, :],
                            KT[:, (c0 + i) * 128:(c0 + i + 1) * 128],
                            qs, start=True, stop=True,
                        )
                    if p in (0, 1):
                        # previous iteration's last rs/av pairs land under
                        # this iteration's first exp latencies
                        flush_tail()
                    if p == 2:
                        flush_epilogue()
                    nc.scalar.activation(
                        PTh[c0 // 8][:, c0 % 8:c0 % 8 + 2, :], ps_s, AF.Exp,
                        scale=SC,
                    )
                    if p >= 2:
                        # two-pair skew: the exp feeding these rs/av matmuls
                        # finished ~2.6us ago, so sem jitter never stalls PE
                        rs_av(c0 - 4)
                        rs_av(c0 - 3)
                pending_tails.extend([
                    lambda rs_av=rs_av: (rs_av(nKV - 4), rs_av(nKV - 3)),
                    lambda rs_av=rs_av: (rs_av(nKV - 2), rs_av(nKV - 1)),
                ])
                pending = (ps_rs, ps_av, h, s4)

            # head 3 of this s4 must fully land before Oproj(s4)
            flush_tail()
            flush_tail()
            flush_epilogue()

            for q4 in range(4):
                pss = quad_psum()
                for o in range(4):
                    for j in range(4):
                        nc.tensor.matmul(
                            pss[j],
                            wor[:, o, (q4 * 4 + j) * 128:(q4 * 4 + j + 1) * 128],
                            OT[:, o, s4 * 512:(s4 + 1) * 512],
                            start=(o == 0), stop=(o == 3), skip_group_check=True,
                        )
                for j in range(4):
                    ys = ystg.tile([P, 512], F16, tag="y")
                    nc.vector.tensor_copy(ys, pss[j])
                    nc.sync.dma_start(
                        out=yt_d[(q4 * 4 + j) * 128:(q4 * 4 + j + 1) * 128,
                                 s4 * 512:(s4 + 1) * 512],
                        in_=ys,
                    )

    nc.compile()
    return nc


_nc = None


def _get_nc():
    global _nc
    if _nc is None:
        _nc = build_nc()
    return _nc


def _pack_act(x):
    # [S, E] fp32 -> transposed slab layout [4(s4), P, nE, 512] fp16
    xt = np.asarray(x, np.float32).T.astype(np.float16)        # [E, S]
    return np.ascontiguousarray(
        xt.reshape(nE, P, 4, 512).transpose(2, 1, 0, 3)
    )


def _pack_w(w, ncols):
    # [E, ncols] fp32 -> [P, nE, ncols] fp16
    wh = np.asarray(w, np.float32).astype(np.float16)
    return np.ascontiguousarray(wh.reshape(nE, P, ncols).transpose(1, 0, 2))


def _make_in_maps(query, key, value, mask, Wq, Wk, Wv, Wo):
    qts = [_pack_act(query[b]) for b in range(B)]
    kts = [_pack_act(key[b]) for b in range(B)]
    vts = [_pack_act(value[b]) for b in range(B)]
    ms = [np.ascontiguousarray(mask[b], dtype=np.float32) for b in range(B)]
    wq_f, wk_f, wv_f, wo_f = (np.asarray(w, np.float32) for w in (Wq, Wk, Wv, Wo))
    in_maps = []
    for c in range(N_CORES):
        b, g = c // 4, c % 4
        wo_slice = wo_f[g * GQ:(g + 1) * GQ, :].astype(np.float16)  # [512, E]
        in_maps.append({
            "qt": qts[b],
            "kt": kts[b],
            "vt": vts[b],
            "m": ms[b],
            "wq": _pack_w(wq_f[:, g * GQ:(g + 1) * GQ], GQ),
            "wk": _pack_w(wk_f[:, g * GK:(g + 1) * GK], GK),
            "wv": _pack_w(wv_f[:, g * GK:(g + 1) * GK], GK),
            "wo": np.ascontiguousarray(
                wo_slice.reshape(4, P, E).transpose(1, 0, 2)
            ),
        })
    return in_maps


def run(query, key, value, mask, Wq, Wk, Wv, Wo, trace=False, trace_kwargs=None):
    from concourse.bass_utils import run_bass_kernel_spmd

    nc = _get_nc()
    in_maps = _make_in_maps(query, key, value, mask, Wq, Wk, Wv, Wo)
    res = run_bass_kernel_spmd(
        nc, in_maps, list(range(N_CORES)), trace=trace, **(trace_kwargs or {})
    )
    out = np.empty((B, S, E), np.float32)
    for b in range(B):
        acc = np.zeros((E, S), np.float32)
        for g in range(4):
            acc += res.results[b * 4 + g]["yt"].astype(np.float32)
        out[b] = acc.T
    return out, res


def kernel(query, key, value, mask, Wq, Wk, Wv, Wo):
    out, _ = run(query, key, value, mask, Wq, Wk, Wv, Wo, trace=False)
    return out
